# revision 1
# baseline (speedup 1.0000x reference)
"""GCNConvRnd kernel for 8 Trainium2 NeuronCores (Bass/Tile).

out = segment_sum((x @ W.T)[src[keep]] * ew[keep], dst[keep], N) + bias

Strategy (dst-sharded, W applied after aggregation):
  * nodes / output sharded 12500 per core; kept edges partitioned by dst shard
  * full x replicated to every core; each core gathers x[src] rows for its own
    edges with the GPSIMD dma_gather custom instruction (int16 indices), so
    src is split into 4 chunks of 25000 rows (chunk = src // 25000, local
    index = src - chunk*25000 fits int16)
  * each core's edges are sorted by dst and packed into "windows" of <=128
    consecutive dst nodes where every (window, chunk) pair holds <= Q*128
    edges; each (window, chunk) is padded to exactly Q blocks of 128 edges
    -> fully static, SPMD-uniform program (B = 4*Q blocks per window)
  * gather streams are chunk-major (all blocks of chunk m contiguous), so a
    handful of large dma_gather calls per chunk move all rows; compute
    consumes blocks window-major at statically known positions
  * per 128-edge block: S_T[e, d] = (iota[e,d] == dstv[e]) * ew[e]  (one DVE
    tensor_scalar), then PE matmul  psum[f, d] += G[e, f].T @ S_T[e, d]
    accumulating over the window's 4*Q blocks
  * per window: PSUM -> SBUF accumulator column w
  * epilogue: out2 = W @ acc (+bias) in 512-wide chunks, DMA to HBM
  * host unpacks windows back to node order
"""

import os
import numpy as np
from contextlib import ExitStack

import concourse.bass as bass
from concourse.bass import AP
import concourse.mybir as mybir
import concourse.tile as tile
from concourse import bacc
from concourse.bass_utils import run_bass_kernel_spmd

N_NODES = 100000
F = 128
P = 128
NC = 8
NPC = N_NODES // NC      # 12500 nodes per core
NCHUNKS = 4
CHUNK = N_NODES // NCHUNKS  # 25000 rows per src chunk (int16-addressable)

# Tunables
Q = int(os.environ.get("GCN_Q", "4"))        # blocks per (window, chunk)
NB = int(os.environ.get("GCN_NB", "8"))     # class-stream blocks per gather call
G_BUFS = int(os.environ.get("GCN_GBUFS", "4"))
S_BUFS = int(os.environ.get("GCN_SBUFS", "6"))
PS_BUFS = int(os.environ.get("GCN_PSBUFS", "4"))
REPS = int(os.environ.get("GCN_REPS", "1"))  # in-NEFF repetitions (timing only)

f32 = mybir.dt.float32
i16 = mybir.dt.int16

_PROGRAM_CACHE: dict = {}


def _preprocess(edge_src, edge_dst, edge_weight, idx_keep, q):
    """Shard kept edges by dst, pack dst windows under per-chunk quotas, and
    emit the static device layout.

    Returns None if quotas are infeasible (a single node overflows a chunk
    quota); caller bumps q.
    """
    src = np.ascontiguousarray(edge_src)[idx_keep].astype(np.int64)
    dst = np.ascontiguousarray(edge_dst)[idx_keep].astype(np.int64)
    ew = np.ascontiguousarray(edge_weight)[idx_keep].astype(np.float32)
    order = np.argsort(dst, kind="stable")
    src, dst, ew = src[order], dst[order], ew[order]
    core_bounds = np.searchsorted(dst, np.arange(NC + 1) * NPC)

    QCAP = q * P
    B = NCHUNKS * q  # compute blocks per window
    percore = []
    for c in range(NC):
        lo, hi = int(core_bounds[c]), int(core_bounds[c + 1])
        dl = dst[lo:hi] - c * NPC
        ch = src[lo:hi] // CHUNK
        # counts per (node, chunk)
        cnts = np.bincount(dl * NCHUNKS + ch, minlength=NPC * NCHUNKS).reshape(
            NPC, NCHUNKS
        )
        # greedy windows under 128-node span + per-chunk quota
        wins = []
        n = 0
        cl = cnts.tolist()
        while n < NPC:
            s = n
            acc = [0, 0, 0, 0]
            while n < NPC and (n - s) < P:
                row = cl[n]
                if any(acc[m] + row[m] > QCAP for m in range(NCHUNKS)):
                    break
                for m in range(NCHUNKS):
                    acc[m] += row[m]
                n += 1
            if n == s:
                return None
            wins.append((s, n))
        percore.append((lo, hi, dl, ch, cnts, wins))

    W_CAP = max(len(pc[5]) for pc in percore)
    W_CAP = -(-W_CAP // 4) * 4  # multiple of 4 -> epilogue chunks of 512
    NBLK = W_CAP * B             # compute blocks per core
    CSB = W_CAP * q              # class-stream blocks per chunk per core
    NIDX = CSB * P               # idxs per chunk stream

    idxbuf = np.zeros((NC, P, NCHUNKS * (NIDX // 16)), np.int16)
    dstv = np.zeros((NC, NBLK * P), np.float32)
    ewv = np.zeros((NC, NBLK * P), np.float32)
    metas = []
    for c, (lo, hi, dl, ch, cnts, wins) in enumerate(percore):
        ne = hi - lo
        # sort core edges by (node, chunk)
        key = dl * NCHUNKS + ch
        o2 = np.argsort(key, kind="stable")
        src_l = src[lo:hi][o2] - ch[o2] * CHUNK
        ew_l = ew[lo:hi][o2]
        dl_l = dl[o2]
        ch_l = ch[o2]
        # seg_start per (node, chunk) in sorted order
        S = np.zeros(NPC * NCHUNKS + 1, np.int64)
        np.cumsum(cnts.ravel(), out=S[1:])
        segstart = S[dl_l * NCHUNKS + ch_l]
        rank = np.arange(ne) - segstart
        # exclusive per-chunk cumsum over nodes
        Cn = np.zeros((NPC + 1, NCHUNKS), np.int64)
        np.cumsum(cnts, axis=0, out=Cn[1:])
        # window id / window start node per node
        win_of = np.zeros(NPC, np.int64)
        wstart = np.zeros(NPC, np.int64)
        for w, (s, e) in enumerate(wins):
            win_of[s:e] = w
            wstart[s:e] = s
        wj = win_of[dl_l]
        swj = wstart[dl_l]
        off_in_seg = Cn[dl_l, ch_l] - Cn[swj, ch_l]
        # slot within chunk stream
        slot = wj * (q * P) + off_in_seg + rank
        # fill idx buffer: chunk stream m, idx i -> partition i%16, col i//16
        cols = slot // 16
        parts = slot % 16
        base_cols = ch_l * (NIDX // 16)
        flat16 = np.zeros((16, NCHUNKS * (NIDX // 16)), np.int16)
        flat16[parts, base_cols + cols] = src_l.astype(np.int16)
        idxbuf[c] = np.tile(flat16, (8, 1))
        # compute-order block arrays
        cb = slot // P          # class block = w*q + qq
        pp = slot % P
        qq = cb % q
        blk = wj * B + ch_l * q + qq
        dv = np.zeros((NBLK, P), np.float32)
        ev = np.zeros((NBLK, P), np.float32)
        dv[blk, pp] = (dl_l - swj).astype(np.float32)
        ev[blk, pp] = ew_l
        dstv[c] = dv.reshape(-1)
        ewv[c] = ev.reshape(-1)
        metas.append(wins)

    dstv = np.ascontiguousarray(
        dstv.reshape(NC, NBLK, P).transpose(0, 2, 1)
    )
    ewv = np.ascontiguousarray(ewv.reshape(NC, NBLK, P).transpose(0, 2, 1))
    return idxbuf, dstv, ewv, metas, W_CAP, NBLK, CSB


def _build_program(W_CAP, q, NBLK, CSB):
    key = (W_CAP, q, NBLK, CSB, NB, G_BUFS, S_BUFS, PS_BUFS, REPS)
    if key in _PROGRAM_CACHE:
        return _PROGRAM_CACHE[key]

    B = NCHUNKS * q
    NIDX = CSB * P
    IDXCOLS = NCHUNKS * (NIDX // 16)

    nc = bacc.Bacc(
        "TRN2",
        target_bir_lowering=False,
        debug=False,
        enable_asserts=False,
        num_devices=NC,
        num_swdge_queues=4,
    )
    x_h = nc.dram_tensor("x", [N_NODES, F], f32, kind="ExternalInput")
    idx_d = nc.dram_tensor("idx", [P, IDXCOLS], i16, kind="ExternalInput").ap()
    dstv_d = nc.dram_tensor("dstv", [P, NBLK], f32, kind="ExternalInput").ap()
    ewv_d = nc.dram_tensor("ewv", [P, NBLK], f32, kind="ExternalInput").ap()
    wt_d = nc.dram_tensor("wt", [P, P], f32, kind="ExternalInput").ap()  # W.T
    bias_d = nc.dram_tensor("biasv", [P, 1], f32, kind="ExternalInput").ap()
    out_d = nc.dram_tensor("out", [P, W_CAP * P], f32, kind="ExternalOutput").ap()

    iota_np = np.broadcast_to(np.arange(P, dtype=np.float32), (P, P)).copy()
    iota_d = nc.inline_tensor(iota_np, "iota").ap()

    NOCHUNK = (W_CAP * P) // 512
    NGC = (CSB + NB - 1) // NB  # gather calls per chunk

    with tile.TileContext(nc) as tc, ExitStack() as ctx:
        const = ctx.enter_context(tc.tile_pool(name="const", bufs=1))
        gpools = [
            ctx.enter_context(tc.tile_pool(name=f"g{m}", bufs=G_BUFS))
            for m in range(NCHUNKS)
        ]
        spool = ctx.enter_context(tc.tile_pool(name="s", bufs=S_BUFS))
        pspool = ctx.enter_context(tc.tile_pool(name="ps", bufs=PS_BUFS, space="PSUM"))
        ps2pool = ctx.enter_context(tc.tile_pool(name="ps2", bufs=2, space="PSUM"))
        stpool = ctx.enter_context(tc.tile_pool(name="st", bufs=2))

        iota_sb = const.tile([P, P], f32)
        nc.sync.dma_start(out=iota_sb[:], in_=iota_d[:])
        wt_sb = const.tile([P, P], f32)
        nc.sync.dma_start(out=wt_sb[:], in_=wt_d[:])
        bias_sb = const.tile([P, 1], f32)
        nc.sync.dma_start(out=bias_sb[:], in_=bias_d[:])
        idx_sb = const.tile([P, IDXCOLS], i16)
        nc.sync.dma_start(out=idx_sb[:], in_=idx_d[:])
        dstv_sb = const.tile([P, NBLK], f32)
        nc.sync.dma_start(out=dstv_sb[:], in_=dstv_d[:])
        ewv_sb = const.tile([P, NBLK], f32)
        nc.sync.dma_start(out=ewv_sb[:], in_=ewv_d[:])
        acc = const.tile([P, W_CAP * P], f32)

        g_tiles = {}

        def body():
            g_tiles.clear()
            for w in range(W_CAP):
                ps = pspool.tile([P, P], f32, space="PSUM")
                for m in range(NCHUNKS):
                    for qq in range(q):
                        blk = w * B + m * q + qq
                        cb = w * q + qq
                        t, col = divmod(cb, NB)
                        g = ensure_gather(m, t)
                        s_t = spool.tile([P, P], f32)
                        nc.vector.tensor_scalar(
                            out=s_t[:],
                            in0=iota_sb[:],
                            scalar1=dstv_sb[:, blk:blk + 1],
                            scalar2=ewv_sb[:, blk:blk + 1],
                            op0=mybir.AluOpType.is_equal,
                            op1=mybir.AluOpType.mult,
                        )
                        first = m == 0 and qq == 0
                        last = m == NCHUNKS - 1 and qq == q - 1
                        nc.tensor.matmul(
                            out=ps[:],
                            lhsT=g[:, col, :],
                            rhs=s_t[:],
                            start=first,
                            stop=last,
                        )
                nc.vector.tensor_copy(out=acc[:, w * P:(w + 1) * P], in_=ps[:])

            for cix in range(NOCHUNK):
                ps2 = ps2pool.tile([P, 512], f32, space="PSUM")
                nc.tensor.matmul(
                    out=ps2[:],
                    lhsT=wt_sb[:],
                    rhs=acc[:, cix * 512:(cix + 1) * 512],
                    start=True,
                    stop=True,
                )
                st = stpool.tile([P, 512], f32)
                nc.vector.tensor_scalar(
                    out=st[:],
                    in0=ps2[:],
                    scalar1=bias_sb[:, 0:1],
                    scalar2=None,
                    op0=mybir.AluOpType.add,
                )
                nc.sync.dma_start(out=out_d[:, cix * 512:(cix + 1) * 512], in_=st[:])

        def ensure_gather(m, t):
            if (m, t) in g_tiles:
                return g_tiles[(m, t)]
            nb = min(NB, CSB - t * NB)
            n_idx = nb * P
            g = gpools[m].tile([P, NB, F], f32)
            nc.gpsimd.dma_gather(
                out_ap=g[:, :nb, :],
                in_ap=AP(x_h, m * CHUNK * P, [(P, CHUNK), (1, P)]),
                idxs_ap=idx_sb[
                    :, m * (NIDX // 16) + t * NB * 8:
                       m * (NIDX // 16) + t * NB * 8 + n_idx // 16
                ],
                num_idxs=n_idx,
                num_idxs_reg=n_idx,
                elem_size=F,
                single_packet=False,
                queue_num=m,
            )
            g_tiles[(m, t)] = g
            return g

        if REPS > 1:
            with tc.For_i(0, REPS, 1):
                body()
        else:
            body()

    nc.compile()
    _PROGRAM_CACHE[key] = nc
    return nc


def _prepare(x, W, bias, edge_src, edge_dst, edge_weight, idx_keep):
    q = Q
    while True:
        pre = _preprocess(edge_src, edge_dst, edge_weight, idx_keep, q)
        if pre is not None:
            break
        q += 1
    idxbuf, dstv, ewv, metas, W_CAP, NBLK, CSB = pre
    nc = _build_program(W_CAP, q, NBLK, CSB)

    x = np.ascontiguousarray(x, dtype=np.float32)
    wt = np.ascontiguousarray(np.asarray(W, dtype=np.float32).T)
    biasv = np.ascontiguousarray(np.asarray(bias, dtype=np.float32).reshape(P, 1))
    in_maps = [
        {
            "x": x,
            "idx": idxbuf[c],
            "dstv": dstv[c],
            "ewv": ewv[c],
            "wt": wt,
            "biasv": biasv,
        }
        for c in range(NC)
    ]
    return nc, in_maps, metas


def _unpack(results, metas):
    out = np.empty((N_NODES, F), np.float32)
    for c in range(NC):
        o = results[c]["out"]  # [P, W_CAP*P], rows = out features
        base = c * NPC
        for w, (s, e) in enumerate(metas[c]):
            out[base + s:base + e, :] = o[:, w * P:w * P + (e - s)].T
    return out


def kernel(x, W, bias, edge_src, edge_dst, edge_weight, idx_keep):
    nc, in_maps, metas = _prepare(
        x, W, bias, edge_src, edge_dst, edge_weight, idx_keep
    )
    res = run_bass_kernel_spmd(nc, in_maps, list(range(NC)))
    return _unpack(res.results, metas)


# --- helpers for test.py (not used by the grading harness) ---------------

def run_traced(x, W, bias, edge_src, edge_dst, edge_weight, idx_keep):
    nc, in_maps, metas = _prepare(
        x, W, bias, edge_src, edge_dst, edge_weight, idx_keep
    )
    res = run_bass_kernel_spmd(nc, in_maps, list(range(NC)), trace=True)
    return _unpack(res.results, metas), res


def run_sim(x, W, bias, edge_src, edge_dst, edge_weight, idx_keep, cores=(0,)):
    from concourse.bass_interp import CoreSim

    nc, in_maps, metas = _prepare(
        x, W, bias, edge_src, edge_dst, edge_weight, idx_keep
    )
    results = []
    for c in cores:
        sim = CoreSim(nc)
        for k, v in in_maps[c].items():
            sim.tensor(k)[:] = v
        sim.simulate()
        results.append({"out": sim.tensor("out").copy()})
    return results, metas, in_maps



# revision 6
# speedup vs baseline: 1.9602x; 1.9602x over previous
"""GCNConvRnd kernel for 8 Trainium2 NeuronCores (Bass/Tile) — v2 (bf16).

out = segment_sum((x @ W.T)[src[keep]] * ew[keep], dst[keep], N) + bias

Strategy (dst-sharded, W applied after aggregation):
  * kept edges deduped on host (idx_keep samples WITH replacement: ~31%
    duplicates merge into edge-weight multiplicity)
  * x cast to bf16 on host and shipped as the gather source: halves gather
    bytes and makes every matmul a 1-cycle/row bf16 matmul
  * nodes / output sharded 12500 per core; kept edges partitioned by dst
  * each core gathers x16[src] rows with GPSIMD dma_gather (int16 indices,
    4 chunks of 25000 rows so local index fits int16)
  * edges sorted by dst, packed into windows of <=128 consecutive dst nodes,
    each (window, chunk) padded to exactly Q blocks of 128 edges -> fully
    static SPMD-uniform program (B = 4*Q blocks per window)
  * per 128-edge block: S_T[e, d] = (iota[e,d] == dstv[e]) * ew[e] via one
    DVE tensor_scalar (bf16 in/out -> 2x mode), then PE bf16 matmul
    psum[f, d] += G[e, f].T @ S_T[e, d] accumulated over the window's blocks
  * per window: PSUM -> bf16 SBUF accumulator column
  * epilogue: out2 = W @ acc (+bias) in 512-wide bf16 matmuls, DMA to HBM
  * host unpacks windows back to node order
"""

import os
import numpy as np
import ml_dtypes
from contextlib import ExitStack

import concourse.bass as bass
from concourse.bass import AP
import concourse.mybir as mybir
import concourse.tile as tile
from concourse import bacc
from concourse.bass_utils import run_bass_kernel_spmd

N_NODES = 100000
N_EDGES = 1600000
F = 128
P = 128
NC = 8
NPC = N_NODES // NC      # 12500 nodes per core
NCHUNKS = 4
CHUNK = N_NODES // NCHUNKS  # 25000 rows per src chunk (int16-addressable)

# Tunables
Q = int(os.environ.get("GCN_Q", "2"))        # blocks per (window, chunk)
NB = int(os.environ.get("GCN_NB", "25"))     # class-stream blocks per gather
G_BUFS = int(os.environ.get("GCN_GBUFS", "3"))
S_BUFS = int(os.environ.get("GCN_SBUFS", "6"))
PS_BUFS = int(os.environ.get("GCN_PSBUFS", "4"))
REPS = int(os.environ.get("GCN_REPS", "1"))  # in-NEFF repetitions (timing)

f32 = mybir.dt.float32
bf16 = mybir.dt.bfloat16
i16 = mybir.dt.int16

_PROGRAM_CACHE: dict = {}


def _dedup(edge_src, edge_dst, edge_weight, idx_keep):
    """Merge duplicate draws of the same edge into a weight multiplier."""
    cnt = np.bincount(np.asarray(idx_keep), minlength=N_EDGES)
    sel = np.nonzero(cnt)[0]
    src = np.asarray(edge_src)[sel].astype(np.int64)
    dst = np.asarray(edge_dst)[sel].astype(np.int64)
    ew = np.asarray(edge_weight)[sel].astype(np.float32) * cnt[sel].astype(
        np.float32
    )
    return src, dst, ew


def _preprocess(edge_src, edge_dst, edge_weight, idx_keep, q):
    """Shard kept (deduped) edges by dst, pack dst windows under per-chunk
    quotas, and emit the static device layout.

    Returns None if quotas are infeasible; caller bumps q.
    """
    src, dst, ew = _dedup(edge_src, edge_dst, edge_weight, idx_keep)
    order = np.argsort(dst, kind="stable")
    src, dst, ew = src[order], dst[order], ew[order]
    core_bounds = np.searchsorted(dst, np.arange(NC + 1) * NPC)

    QCAP = q * P
    B = NCHUNKS * q  # compute blocks per window
    percore = []
    for c in range(NC):
        lo, hi = int(core_bounds[c]), int(core_bounds[c + 1])
        dl = dst[lo:hi] - c * NPC
        ch = src[lo:hi] // CHUNK
        cnts = np.bincount(dl * NCHUNKS + ch, minlength=NPC * NCHUNKS).reshape(
            NPC, NCHUNKS
        )
        wins = []
        n = 0
        cl = cnts.tolist()
        while n < NPC:
            s = n
            acc = [0, 0, 0, 0]
            while n < NPC and (n - s) < P:
                row = cl[n]
                if any(acc[m] + row[m] > QCAP for m in range(NCHUNKS)):
                    break
                for m in range(NCHUNKS):
                    acc[m] += row[m]
                n += 1
            if n == s:
                return None
            wins.append((s, n))
        percore.append((lo, hi, dl, ch, cnts, wins))

    W_CAP = max(len(pc[5]) for pc in percore)
    W_CAP = -(-W_CAP // 4) * 4  # multiple of 4 -> epilogue chunks of 512
    NBLK = W_CAP * B             # compute blocks per core
    CSB = W_CAP * q              # class-stream blocks per chunk per core
    NIDX = CSB * P               # idxs per chunk stream

    idxbuf = np.zeros((NC, P, NCHUNKS * (NIDX // 16)), np.int16)
    dstv = np.zeros((NC, NBLK * P), np.float32)
    ewv = np.zeros((NC, NBLK * P), np.float32)
    metas = []
    for c, (lo, hi, dl, ch, cnts, wins) in enumerate(percore):
        ne = hi - lo
        key = dl * NCHUNKS + ch
        o2 = np.argsort(key, kind="stable")
        src_l = src[lo:hi][o2] - ch[o2] * CHUNK
        ew_l = ew[lo:hi][o2]
        dl_l = dl[o2]
        ch_l = ch[o2]
        S = np.zeros(NPC * NCHUNKS + 1, np.int64)
        np.cumsum(cnts.ravel(), out=S[1:])
        segstart = S[dl_l * NCHUNKS + ch_l]
        rank = np.arange(ne) - segstart
        Cn = np.zeros((NPC + 1, NCHUNKS), np.int64)
        np.cumsum(cnts, axis=0, out=Cn[1:])
        win_of = np.zeros(NPC, np.int64)
        wstart = np.zeros(NPC, np.int64)
        for w, (s, e) in enumerate(wins):
            win_of[s:e] = w
            wstart[s:e] = s
        wj = win_of[dl_l]
        swj = wstart[dl_l]
        off_in_seg = Cn[dl_l, ch_l] - Cn[swj, ch_l]
        slot = wj * (q * P) + off_in_seg + rank
        cols = slot // 16
        parts = slot % 16
        base_cols = ch_l * (NIDX // 16)
        flat16 = np.zeros((16, NCHUNKS * (NIDX // 16)), np.int16)
        flat16[parts, base_cols + cols] = src_l.astype(np.int16)
        idxbuf[c] = np.tile(flat16, (8, 1))
        cb = slot // P
        pp = slot % P
        qq = cb % q
        blk = wj * B + ch_l * q + qq
        dv = np.zeros((NBLK, P), np.float32)
        ev = np.zeros((NBLK, P), np.float32)
        dv[blk, pp] = (dl_l - swj).astype(np.float32)
        ev[blk, pp] = ew_l
        dstv[c] = dv.reshape(-1)
        ewv[c] = ev.reshape(-1)
        metas.append(wins)

    dstv = np.ascontiguousarray(dstv.reshape(NC, NBLK, P).transpose(0, 2, 1))
    ewv = np.ascontiguousarray(ewv.reshape(NC, NBLK, P).transpose(0, 2, 1))
    return idxbuf, dstv, ewv, metas, W_CAP, NBLK, CSB


def _build_program(W_CAP, q, NBLK, CSB):
    key = (W_CAP, q, NBLK, CSB, NB, G_BUFS, S_BUFS, PS_BUFS, REPS)
    if key in _PROGRAM_CACHE:
        return _PROGRAM_CACHE[key]

    B = NCHUNKS * q
    NIDX = CSB * P
    IDXCOLS = NCHUNKS * (NIDX // 16)

    nc = bacc.Bacc(
        "TRN2",
        target_bir_lowering=False,
        debug=False,
        enable_asserts=False,
        num_devices=NC,
        num_swdge_queues=4,
    )
    x_h = nc.dram_tensor("x16", [N_NODES, F], bf16, kind="ExternalInput")
    idx_d = nc.dram_tensor("idx", [P, IDXCOLS], i16, kind="ExternalInput").ap()
    dstv_d = nc.dram_tensor("dstv", [P, NBLK], f32, kind="ExternalInput").ap()
    ewv_d = nc.dram_tensor("ewv", [P, NBLK], f32, kind="ExternalInput").ap()
    wt_d = nc.dram_tensor("wt", [P, P], bf16, kind="ExternalInput").ap()  # W.T
    bias_d = nc.dram_tensor("biasv", [P, 1], f32, kind="ExternalInput").ap()
    out_d = nc.dram_tensor("out", [P, W_CAP * P], f32, kind="ExternalOutput").ap()

    iota_np = np.broadcast_to(
        np.arange(P, dtype=np.float32), (P, P)
    ).astype(ml_dtypes.bfloat16)
    iota_d = nc.inline_tensor(iota_np, "iota").ap()

    NOCHUNK = (W_CAP * P) // 512
    NGC = (CSB + NB - 1) // NB  # gather calls per chunk

    with tile.TileContext(nc) as tc, ExitStack() as ctx:
        const = ctx.enter_context(tc.tile_pool(name="const", bufs=1))
        gpools = [
            ctx.enter_context(tc.tile_pool(name=f"g{m}", bufs=G_BUFS))
            for m in range(NCHUNKS)
        ]
        spool = ctx.enter_context(tc.tile_pool(name="s", bufs=S_BUFS))
        pspool = ctx.enter_context(tc.tile_pool(name="ps", bufs=PS_BUFS, space="PSUM"))
        ps2pool = ctx.enter_context(tc.tile_pool(name="ps2", bufs=2, space="PSUM"))
        stpool = ctx.enter_context(tc.tile_pool(name="st", bufs=2))

        iota_sb = const.tile([P, P], bf16)
        nc.sync.dma_start(out=iota_sb[:], in_=iota_d[:])
        wt_sb = const.tile([P, P], bf16)
        nc.sync.dma_start(out=wt_sb[:], in_=wt_d[:])
        bias_sb = const.tile([P, 1], f32)
        nc.sync.dma_start(out=bias_sb[:], in_=bias_d[:])
        idx_sb = const.tile([P, IDXCOLS], i16)
        nc.sync.dma_start(out=idx_sb[:], in_=idx_d[:])
        dstv_sb = const.tile([P, NBLK], f32)
        nc.sync.dma_start(out=dstv_sb[:], in_=dstv_d[:])
        ewv_sb = const.tile([P, NBLK], f32)
        nc.sync.dma_start(out=ewv_sb[:], in_=ewv_d[:])
        acc = const.tile([P, W_CAP * P], bf16)

        g_tiles = {}

        def body():
            g_tiles.clear()
            for w in range(W_CAP):
                ps = pspool.tile([P, P], f32, space="PSUM")
                for m in range(NCHUNKS):
                    for qq in range(q):
                        blk = w * B + m * q + qq
                        cb = w * q + qq
                        t, col = divmod(cb, NB)
                        g = ensure_gather(m, t)
                        s_t = spool.tile([P, P], bf16)
                        nc.vector.tensor_scalar(
                            out=s_t[:],
                            in0=iota_sb[:],
                            scalar1=dstv_sb[:, blk:blk + 1],
                            scalar2=ewv_sb[:, blk:blk + 1],
                            op0=mybir.AluOpType.is_equal,
                            op1=mybir.AluOpType.mult,
                        )
                        first = m == 0 and qq == 0
                        last = m == NCHUNKS - 1 and qq == q - 1
                        nc.tensor.matmul(
                            out=ps[:],
                            lhsT=g[:, col, :],
                            rhs=s_t[:],
                            start=first,
                            stop=last,
                        )
                nc.scalar.copy(out=acc[:, w * P:(w + 1) * P], in_=ps[:])

            for cix in range(NOCHUNK):
                ps2 = ps2pool.tile([P, 512], f32, space="PSUM")
                nc.tensor.matmul(
                    out=ps2[:],
                    lhsT=wt_sb[:],
                    rhs=acc[:, cix * 512:(cix + 1) * 512],
                    start=True,
                    stop=True,
                )
                st = stpool.tile([P, 512], f32)
                nc.scalar.add(out=st[:], in_=ps2[:], add=bias_sb[:, 0:1])
                nc.sync.dma_start(out=out_d[:, cix * 512:(cix + 1) * 512], in_=st[:])

        def ensure_gather(m, t):
            if (m, t) in g_tiles:
                return g_tiles[(m, t)]
            nb = min(NB, CSB - t * NB)
            n_idx = nb * P
            g = gpools[m].tile([P, NB, F], bf16)
            nc.gpsimd.dma_gather(
                out_ap=g[:, :nb, :],
                in_ap=AP(x_h, m * CHUNK * F, [(F, CHUNK), (1, F)]),
                idxs_ap=idx_sb[
                    :, m * (NIDX // 16) + t * NB * 8:
                       m * (NIDX // 16) + t * NB * 8 + n_idx // 16
                ],
                num_idxs=n_idx,
                num_idxs_reg=n_idx,
                elem_size=F,
                single_packet=False,
                queue_num=m,
            )
            g_tiles[(m, t)] = g
            return g

        if REPS > 1:
            with tc.For_i(0, REPS, 1):
                body()
        else:
            body()

    nc.compile()
    _PROGRAM_CACHE[key] = nc
    return nc


def _prepare(x, W, bias, edge_src, edge_dst, edge_weight, idx_keep):
    q = Q
    while True:
        pre = _preprocess(edge_src, edge_dst, edge_weight, idx_keep, q)
        if pre is not None:
            break
        q += 1
    idxbuf, dstv, ewv, metas, W_CAP, NBLK, CSB = pre
    nc = _build_program(W_CAP, q, NBLK, CSB)

    x16 = np.ascontiguousarray(np.asarray(x, dtype=np.float32)).astype(
        ml_dtypes.bfloat16
    )
    wt = np.ascontiguousarray(
        np.asarray(W, dtype=np.float32).T
    ).astype(ml_dtypes.bfloat16)
    biasv = np.ascontiguousarray(np.asarray(bias, dtype=np.float32).reshape(P, 1))
    in_maps = [
        {
            "x16": x16,
            "idx": idxbuf[c],
            "dstv": dstv[c],
            "ewv": ewv[c],
            "wt": wt,
            "biasv": biasv,
        }
        for c in range(NC)
    ]
    return nc, in_maps, metas


def _unpack(results, metas):
    out = np.empty((N_NODES, F), np.float32)
    for c in range(NC):
        o = results[c]["out"]  # [P, W_CAP*P], rows = out features
        base = c * NPC
        for w, (s, e) in enumerate(metas[c]):
            out[base + s:base + e, :] = o[:, w * P:w * P + (e - s)].T
    return out


def kernel(x, W, bias, edge_src, edge_dst, edge_weight, idx_keep):
    nc, in_maps, metas = _prepare(
        x, W, bias, edge_src, edge_dst, edge_weight, idx_keep
    )
    res = run_bass_kernel_spmd(nc, in_maps, list(range(NC)))
    return _unpack(res.results, metas)


# --- helpers for test.py (not used by the grading harness) ---------------

def run_traced(x, W, bias, edge_src, edge_dst, edge_weight, idx_keep):
    nc, in_maps, metas = _prepare(
        x, W, bias, edge_src, edge_dst, edge_weight, idx_keep
    )
    res = run_bass_kernel_spmd(nc, in_maps, list(range(NC)), trace=True)
    return _unpack(res.results, metas), res


def run_sim(x, W, bias, edge_src, edge_dst, edge_weight, idx_keep, cores=(0,)):
    from concourse.bass_interp import CoreSim

    nc, in_maps, metas = _prepare(
        x, W, bias, edge_src, edge_dst, edge_weight, idx_keep
    )
    results = []
    for c in cores:
        sim = CoreSim(nc)
        for k, v in in_maps[c].items():
            sim.tensor(k)[:] = v
        sim.simulate()
        results.append({"out": sim.tensor("out").copy()})
    return results, metas, in_maps


# revision 41
# speedup vs baseline: 3.1072x; 1.5851x over previous
"""GCNConvRnd kernel for 8 Trainium2 NeuronCores (Bass/Tile) — v2 (bf16).

out = segment_sum((x @ W.T)[src[keep]] * ew[keep], dst[keep], N) + bias

Strategy (dst-sharded, W applied after aggregation):
  * kept edges deduped on host (idx_keep samples WITH replacement: ~31%
    duplicates merge into edge-weight multiplicity)
  * x cast to bf16 on host and shipped as the gather source: halves gather
    bytes and makes every matmul a 1-cycle/row bf16 matmul
  * nodes / output sharded 12500 per core; kept edges partitioned by dst
  * each core gathers x16[src] rows with GPSIMD dma_gather (int16 indices,
    4 chunks of 25000 rows so local index fits int16)
  * edges sorted by dst, packed into windows of <=128 consecutive dst nodes,
    each (window, chunk) padded to exactly Q blocks of 128 edges -> fully
    static SPMD-uniform program (B = 4*Q blocks per window)
  * per 128-edge block: S_T[e, d] = (iota[e,d] == dstv[e]) * ew[e] via one
    DVE tensor_scalar (bf16 in/out -> 2x mode), then PE bf16 matmul
    psum[f, d] += G[e, f].T @ S_T[e, d] accumulated over the window's blocks
  * per window: PSUM -> bf16 SBUF accumulator column
  * epilogue: out2 = W @ acc (+bias) in 512-wide bf16 matmuls, DMA to HBM
  * host unpacks windows back to node order
"""

import os
import numpy as np
import ml_dtypes
from contextlib import ExitStack

import concourse.bass as bass
from concourse.bass import AP
import concourse.mybir as mybir
import concourse.tile as tile
from concourse import bacc
from concourse.bass_utils import run_bass_kernel_spmd

N_NODES = 100000
N_EDGES = 1600000
F = 128
P = 128
NC = 8
NPC = N_NODES // NC      # 12500 nodes per core
NCHUNKS = 4
CHUNK = N_NODES // NCHUNKS  # 25000 rows per src chunk (int16-addressable)

# Tunables
Q = int(os.environ.get("GCN_Q", "2"))        # blocks per (window, chunk)
NB = int(os.environ.get("GCN_NB", "25"))     # class-stream blocks per gather
G_BUFS = int(os.environ.get("GCN_GBUFS", "3"))
S_BUFS = int(os.environ.get("GCN_SBUFS", "6"))
PS_BUFS = int(os.environ.get("GCN_PSBUFS", "4"))
REPS = int(os.environ.get("GCN_REPS", "1"))  # in-NEFF repetitions (timing)
SKIP = os.environ.get("GCN_SKIP", "")        # '', 'gather', or 'compute'
GDT = os.environ.get("GCN_GDT", "bf16")      # gather dtype: 'bf16' | 'f32'
SP_PKT = os.environ.get("GCN_SP", "0") == "1"  # dma_gather single_packet
HALF = os.environ.get("GCN_HALF", "0") == "1"  # diag: half descs, 2x elem
STMODE = os.environ.get("GCN_STMODE", "batch")  # 'batch' | 'scalar'
# every ACTSPLIT-th window builds S_T on the Activation engine (0 = off)
ACTSPLIT = int(os.environ.get("GCN_ACTSPLIT", "3"))

f32 = mybir.dt.float32
bf16 = mybir.dt.bfloat16
i16 = mybir.dt.int16

_PROGRAM_CACHE: dict = {}


def _dedup(edge_src, edge_dst, edge_weight, idx_keep):
    """Merge duplicate draws of the same edge into a weight multiplier."""
    cnt = np.bincount(np.asarray(idx_keep), minlength=N_EDGES)
    sel = np.nonzero(cnt)[0]
    src = np.asarray(edge_src)[sel].astype(np.int64)
    dst = np.asarray(edge_dst)[sel].astype(np.int64)
    ew = np.asarray(edge_weight)[sel].astype(np.float32) * cnt[sel].astype(
        np.float32
    )
    return src, dst, ew


def _preprocess(edge_src, edge_dst, edge_weight, idx_keep, q):
    """Shard kept (deduped) edges by dst, pack dst windows under per-chunk
    quotas, and emit the static device layout.

    Returns None if quotas are infeasible; caller bumps q.
    """
    src, dst, ew = _dedup(edge_src, edge_dst, edge_weight, idx_keep)
    order = np.argsort(dst, kind="stable")
    src, dst, ew = src[order], dst[order], ew[order]
    core_bounds = np.searchsorted(dst, np.arange(NC + 1) * NPC)

    QCAP = q * P
    B = NCHUNKS * q  # compute blocks per window
    percore = []
    for c in range(NC):
        lo, hi = int(core_bounds[c]), int(core_bounds[c + 1])
        dl = dst[lo:hi] - c * NPC
        ch = src[lo:hi] // CHUNK
        cnts = np.bincount(dl * NCHUNKS + ch, minlength=NPC * NCHUNKS).reshape(
            NPC, NCHUNKS
        )
        wins = []
        n = 0
        cl = cnts.tolist()
        while n < NPC:
            s = n
            acc = [0, 0, 0, 0]
            while n < NPC and (n - s) < P:
                row = cl[n]
                if any(acc[m] + row[m] > QCAP for m in range(NCHUNKS)):
                    break
                for m in range(NCHUNKS):
                    acc[m] += row[m]
                n += 1
            if n == s:
                return None
            wins.append((s, n))
        percore.append((lo, hi, dl, ch, cnts, wins))

    W_CAP = max(len(pc[5]) for pc in percore)
    W_CAP = -(-W_CAP // 4) * 4  # multiple of 4 -> epilogue chunks of 512
    NBLK = W_CAP * B             # compute blocks per core
    CSB = W_CAP * q              # class-stream blocks per chunk per core
    NIDX = CSB * P               # idxs per chunk stream

    idxbuf = np.zeros((NC, P, NCHUNKS * (NIDX // 16)), np.int16)
    dstv = np.zeros((NC, NBLK * P), np.float32)
    ewv = np.zeros((NC, NBLK * P), np.float32)
    metas = []
    for c, (lo, hi, dl, ch, cnts, wins) in enumerate(percore):
        ne = hi - lo
        key = dl * NCHUNKS + ch
        o2 = np.argsort(key, kind="stable")
        src_l = src[lo:hi][o2] - ch[o2] * CHUNK
        ew_l = ew[lo:hi][o2]
        dl_l = dl[o2]
        ch_l = ch[o2]
        S = np.zeros(NPC * NCHUNKS + 1, np.int64)
        np.cumsum(cnts.ravel(), out=S[1:])
        segstart = S[dl_l * NCHUNKS + ch_l]
        rank = np.arange(ne) - segstart
        Cn = np.zeros((NPC + 1, NCHUNKS), np.int64)
        np.cumsum(cnts, axis=0, out=Cn[1:])
        win_of = np.zeros(NPC, np.int64)
        wstart = np.zeros(NPC, np.int64)
        for w, (s, e) in enumerate(wins):
            win_of[s:e] = w
            wstart[s:e] = s
        wj = win_of[dl_l]
        swj = wstart[dl_l]
        off_in_seg = Cn[dl_l, ch_l] - Cn[swj, ch_l]
        slot = wj * (q * P) + off_in_seg + rank
        cols = slot // 16
        parts = slot % 16
        base_cols = ch_l * (NIDX // 16)
        flat16 = np.zeros((16, NCHUNKS * (NIDX // 16)), np.int16)
        flat16[parts, base_cols + cols] = src_l.astype(np.int16)
        idxbuf[c] = np.tile(flat16, (8, 1))
        cb = slot // P
        pp = slot % P
        qq = cb % q
        blk = wj * B + ch_l * q + qq
        dv = np.zeros((NBLK, P), np.float32)
        ev = np.zeros((NBLK, P), np.float32)
        dv[blk, pp] = (dl_l - swj).astype(np.float32)
        ev[blk, pp] = ew_l
        dstv[c] = dv.reshape(-1)
        ewv[c] = ev.reshape(-1)
        metas.append(wins)

    dstv = np.ascontiguousarray(dstv.reshape(NC, NBLK, P).transpose(0, 2, 1))
    ewv = np.ascontiguousarray(ewv.reshape(NC, NBLK, P).transpose(0, 2, 1))
    return idxbuf, dstv, ewv, metas, W_CAP, NBLK, CSB


def _build_program(W_CAP, q, NBLK, CSB):
    key = (W_CAP, q, NBLK, CSB, NB, G_BUFS, S_BUFS, PS_BUFS, REPS, SKIP, GDT,
           SP_PKT, HALF, STMODE, ACTSPLIT)
    if key in _PROGRAM_CACHE:
        return _PROGRAM_CACHE[key]

    B = NCHUNKS * q
    NIDX = CSB * P
    IDXCOLS = NCHUNKS * (NIDX // 16)

    nc = bacc.Bacc(
        "TRN2",
        target_bir_lowering=False,
        debug=False,
        enable_asserts=False,
        num_devices=NC,
        num_swdge_queues=4,
    )
    gdt = bf16 if GDT == "bf16" else f32
    x_h = nc.dram_tensor("x16", [N_NODES, F], gdt, kind="ExternalInput")
    idx_d = nc.dram_tensor("idx", [P, IDXCOLS], i16, kind="ExternalInput").ap()
    sdt = bf16 if STMODE == "batch" else f32
    dstv_d = nc.dram_tensor("dstv", [P, NBLK], sdt, kind="ExternalInput").ap()
    ewv_d = nc.dram_tensor("ewv", [P, NBLK], sdt, kind="ExternalInput").ap()
    if ACTSPLIT:
        # fp32 per-partition scale/bias columns for the ACT-engine one-hot:
        # s_t = Relu(ew - ew*|iota - dst|)
        dstn_d = nc.dram_tensor("dstn", [P, NBLK], f32, kind="ExternalInput").ap()
        ewf_d = nc.dram_tensor("ewf", [P, NBLK], f32, kind="ExternalInput").ap()
        ewn_d = nc.dram_tensor("ewn", [P, NBLK], f32, kind="ExternalInput").ap()
    wt_d = nc.dram_tensor("wt", [P, P], bf16, kind="ExternalInput").ap()  # W.T
    bias_d = nc.dram_tensor("biasv", [P, 1], f32, kind="ExternalInput").ap()
    out_d = nc.dram_tensor("out", [P, W_CAP * P], f32, kind="ExternalOutput").ap()

    iota_np = np.broadcast_to(
        np.arange(P, dtype=np.float32), (P, P)
    ).astype(ml_dtypes.bfloat16)
    iota_d = nc.inline_tensor(iota_np, "iota").ap()

    NOCHUNK = (W_CAP * P) // 512
    NGC = (CSB + NB - 1) // NB  # gather calls per chunk

    with tile.TileContext(nc) as tc, ExitStack() as ctx:
        const = ctx.enter_context(tc.tile_pool(name="const", bufs=1))
        gpools = [
            ctx.enter_context(tc.tile_pool(name=f"g{m}", bufs=G_BUFS))
            for m in range(NCHUNKS)
        ]
        spool = ctx.enter_context(tc.tile_pool(name="s", bufs=S_BUFS))
        pspool = ctx.enter_context(tc.tile_pool(name="ps", bufs=PS_BUFS, space="PSUM"))
        ps2pool = ctx.enter_context(tc.tile_pool(name="ps2", bufs=2, space="PSUM"))
        stpool = ctx.enter_context(tc.tile_pool(name="st", bufs=2))

        iota_sb = const.tile([P, P], bf16)
        nc.sync.dma_start(out=iota_sb[:], in_=iota_d[:])
        wt_sb = const.tile([P, P], bf16)
        nc.sync.dma_start(out=wt_sb[:], in_=wt_d[:])
        bias_sb = const.tile([P, 1], f32)
        nc.sync.dma_start(out=bias_sb[:], in_=bias_d[:])
        idx_sb = const.tile([P, IDXCOLS], i16)
        nc.sync.dma_start(out=idx_sb[:], in_=idx_d[:])
        dstv_sb = const.tile([P, NBLK], sdt)
        nc.sync.dma_start(out=dstv_sb[:], in_=dstv_d[:])
        ewv_sb = const.tile([P, NBLK], sdt)
        nc.sync.dma_start(out=ewv_sb[:], in_=ewv_d[:])
        if ACTSPLIT:
            dstn_sb = const.tile([P, NBLK], f32)
            nc.sync.dma_start(out=dstn_sb[:], in_=dstn_d[:])
            ewf_sb = const.tile([P, NBLK], f32)
            nc.sync.dma_start(out=ewf_sb[:], in_=ewf_d[:])
            ewn_sb = const.tile([P, NBLK], f32)
            nc.sync.dma_start(out=ewn_sb[:], in_=ewn_d[:])
        acc = const.tile([P, W_CAP * P], bf16)

        g_tiles = {}

        def body():
            g_tiles.clear()
            if SKIP == "compute":
                for t in range(NGC):  # t-major: keep all 4 queues busy
                    for m in range(NCHUNKS):
                        ensure_gather(m, t)
                return
            for w in range(W_CAP):
                ps = pspool.tile([P, P], f32, space="PSUM")
                act_win = ACTSPLIT and (w % ACTSPLIT == ACTSPLIT - 1)
                if act_win:
                    pass  # per-block ACT build below
                elif STMODE == "batch":
                    # one S_T strip for all B blocks of the window:
                    # tmp = (iota bcast) == (dstv bcast); s_t = tmp * ew
                    s_t = spool.tile([P, B * P], bf16)
                    tmp = spool.tile([P, B * P], bf16)
                    iota_b = AP(
                        iota_sb.tensor,
                        iota_sb[:].offset,
                        [iota_sb[:].ap[0], (0, B), (1, P)],
                    )
                    dstv_b = AP(
                        dstv_sb.tensor,
                        dstv_sb[:, w * B:(w + 1) * B].offset,
                        [dstv_sb[:].ap[0], (1, B), (0, P)],
                    )
                    ewv_b = AP(
                        ewv_sb.tensor,
                        ewv_sb[:, w * B:(w + 1) * B].offset,
                        [ewv_sb[:].ap[0], (1, B), (0, P)],
                    )
                    nc.vector.tensor_tensor(
                        out=tmp[:], in0=iota_b, in1=dstv_b,
                        op=mybir.AluOpType.is_equal,
                    )
                    nc.vector.tensor_tensor(
                        out=s_t[:], in0=tmp[:], in1=ewv_b,
                        op=mybir.AluOpType.mult,
                    )
                for m in range(NCHUNKS):
                    for qq in range(q):
                        blk = w * B + m * q + qq
                        bl = m * q + qq  # block index within window
                        cb = w * q + qq
                        t, col = divmod(cb, NB)
                        g = ensure_gather(m, t)
                        if act_win:
                            u = spool.tile([P, P], bf16)
                            nc.scalar.activation(
                                out=u[:], in_=iota_sb[:],
                                func=mybir.ActivationFunctionType.Abs,
                                bias=dstn_sb[:, blk:blk + 1], scale=1.0,
                            )
                            s1 = spool.tile([P, P], bf16)
                            nc.scalar.activation(
                                out=s1[:], in_=u[:],
                                func=mybir.ActivationFunctionType.Relu,
                                bias=ewf_sb[:, blk:blk + 1],
                                scale=ewn_sb[:, blk:blk + 1],
                            )
                            rhs = s1[:]
                        elif STMODE == "batch":
                            rhs = s_t[:, bl * P:(bl + 1) * P]
                        else:
                            s1 = spool.tile([P, P], bf16)
                            nc.vector.tensor_scalar(
                                out=s1[:],
                                in0=iota_sb[:],
                                scalar1=dstv_sb[:, blk:blk + 1],
                                scalar2=ewv_sb[:, blk:blk + 1],
                                op0=mybir.AluOpType.is_equal,
                                op1=mybir.AluOpType.mult,
                            )
                            rhs = s1[:]
                        first = m == 0 and qq == 0
                        last = m == NCHUNKS - 1 and qq == q - 1
                        nc.tensor.matmul(
                            out=ps[:],
                            lhsT=g[:, col, :],
                            rhs=rhs,
                            start=first,
                            stop=last,
                        )
                if act_win:  # keep ACT free on its windows
                    nc.vector.tensor_copy(
                        out=acc[:, w * P:(w + 1) * P], in_=ps[:]
                    )
                else:
                    nc.scalar.copy(out=acc[:, w * P:(w + 1) * P], in_=ps[:])
            epilogue()

        def epilogue():
            for cix in range(NOCHUNK):
                ps2 = ps2pool.tile([P, 512], f32, space="PSUM")
                nc.tensor.matmul(
                    out=ps2[:],
                    lhsT=wt_sb[:],
                    rhs=acc[:, cix * 512:(cix + 1) * 512],
                    start=True,
                    stop=True,
                )
                st = stpool.tile([P, 512], f32)
                nc.scalar.add(out=st[:], in_=ps2[:], add=bias_sb[:, 0:1])
                nc.sync.dma_start(out=out_d[:, cix * 512:(cix + 1) * 512], in_=st[:])

        def ensure_gather(m, t):
            if (m, t) in g_tiles:
                return g_tiles[(m, t)]
            nb = min(NB, CSB - t * NB)
            n_idx = nb * P
            g = gpools[m].tile([P, NB, F], gdt)
            if SKIP == "gather":
                # sequential-stream stand-in write: keeps the tile written
                # (framework requirement) at streaming DMA cost, no descgen
                nc.sync.dma_start(
                    out=g[:, :nb, :],
                    in_=AP(x_h, m * CHUNK * F,
                           [(F, P), (F * P, nb), (1, F)]),
                )
                g_tiles[(m, t)] = g
                return g
            if HALF:
                # diagnostic only (wrong data): same bytes, half descriptors
                g2 = gpools[m].tile([P, NB // 2, 2 * F], gdt)
                g_tiles[(m, t)] = g2
                nc.gpsimd.dma_gather(
                    out_ap=g2[:, :nb // 2, :],
                    in_ap=AP(x_h, 0, [(2 * F, CHUNK), (1, 2 * F)]),
                    idxs_ap=idx_sb[
                        :, m * (NIDX // 16) + t * NB * 8:
                           m * (NIDX // 16) + t * NB * 8 + n_idx // 32
                    ],
                    num_idxs=n_idx // 2,
                    num_idxs_reg=n_idx // 2,
                    elem_size=2 * F,
                    single_packet=SP_PKT,
                    queue_num=m,
                )
                return g2
            else:
                nc.gpsimd.dma_gather(
                    out_ap=g[:, :nb, :],
                    in_ap=AP(x_h, m * CHUNK * F, [(F, CHUNK), (1, F)]),
                    idxs_ap=idx_sb[
                        :, m * (NIDX // 16) + t * NB * 8:
                           m * (NIDX // 16) + t * NB * 8 + n_idx // 16
                    ],
                    num_idxs=n_idx,
                    num_idxs_reg=n_idx,
                    elem_size=F,
                    single_packet=SP_PKT,
                    queue_num=m,
                )
            g_tiles[(m, t)] = g
            return g

        if REPS > 1:
            with tc.For_i(0, REPS, 1):
                body()
        else:
            body()

    nc.compile()
    _PROGRAM_CACHE[key] = nc
    return nc


def _prepare(x, W, bias, edge_src, edge_dst, edge_weight, idx_keep):
    q = Q
    while True:
        pre = _preprocess(edge_src, edge_dst, edge_weight, idx_keep, q)
        if pre is not None:
            break
        q += 1
    idxbuf, dstv, ewv, metas, W_CAP, NBLK, CSB = pre
    nc = _build_program(W_CAP, q, NBLK, CSB)

    xf = np.ascontiguousarray(np.asarray(x, dtype=np.float32))
    x16 = xf.astype(ml_dtypes.bfloat16) if GDT == "bf16" else xf
    wt = np.ascontiguousarray(
        np.asarray(W, dtype=np.float32).T
    ).astype(ml_dtypes.bfloat16)
    biasv = np.ascontiguousarray(np.asarray(bias, dtype=np.float32).reshape(P, 1))
    dstv32, ewv32 = dstv, ewv
    if STMODE == "batch":
        dstv = dstv.astype(ml_dtypes.bfloat16)
        ewv = ewv.astype(ml_dtypes.bfloat16)
    in_maps = [
        {
            "x16": x16,
            "idx": idxbuf[c],
            "dstv": dstv[c],
            "ewv": ewv[c],
            "wt": wt,
            "biasv": biasv,
        }
        for c in range(NC)
    ]
    if ACTSPLIT:
        for c in range(NC):
            in_maps[c]["dstn"] = -dstv32[c]
            in_maps[c]["ewf"] = ewv32[c]
            in_maps[c]["ewn"] = -ewv32[c]
    return nc, in_maps, metas


def _unpack(results, metas):
    out = np.empty((N_NODES, F), np.float32)
    for c in range(NC):
        o = results[c]["out"]  # [P, W_CAP*P], rows = out features
        base = c * NPC
        for w, (s, e) in enumerate(metas[c]):
            out[base + s:base + e, :] = o[:, w * P:w * P + (e - s)].T
    return out


def kernel(x, W, bias, edge_src, edge_dst, edge_weight, idx_keep):
    nc, in_maps, metas = _prepare(
        x, W, bias, edge_src, edge_dst, edge_weight, idx_keep
    )
    res = run_bass_kernel_spmd(nc, in_maps, list(range(NC)))
    return _unpack(res.results, metas)


# --- helpers for test.py (not used by the grading harness) ---------------

def run_traced(x, W, bias, edge_src, edge_dst, edge_weight, idx_keep):
    nc, in_maps, metas = _prepare(
        x, W, bias, edge_src, edge_dst, edge_weight, idx_keep
    )
    res = run_bass_kernel_spmd(nc, in_maps, list(range(NC)), trace=True)
    return _unpack(res.results, metas), res


def run_sim(x, W, bias, edge_src, edge_dst, edge_weight, idx_keep, cores=(0,)):
    from concourse.bass_interp import CoreSim

    nc, in_maps, metas = _prepare(
        x, W, bias, edge_src, edge_dst, edge_weight, idx_keep
    )
    results = []
    for c in cores:
        sim = CoreSim(nc)
        for k, v in in_maps[c].items():
            sim.tensor(k)[:] = v
        sim.simulate()
        results.append({"out": sim.tensor("out").copy()})
    return results, metas, in_maps


# revision 55
# speedup vs baseline: 3.2622x; 1.0499x over previous
"""GCNConvRnd kernel for 8 Trainium2 NeuronCores (Bass/Tile) — v2 (bf16).

out = segment_sum((x @ W.T)[src[keep]] * ew[keep], dst[keep], N) + bias

Strategy (dst-sharded, W applied after aggregation):
  * kept edges deduped on host (idx_keep samples WITH replacement: ~31%
    duplicates merge into edge-weight multiplicity)
  * x cast to bf16 on host and shipped as the gather source: halves gather
    bytes and makes every matmul a 1-cycle/row bf16 matmul
  * nodes / output sharded 12500 per core; kept edges partitioned by dst
  * each core gathers x16[src] rows with GPSIMD dma_gather (int16 indices,
    4 chunks of 25000 rows so local index fits int16)
  * edges sorted by dst, packed into windows of <=128 consecutive dst nodes,
    each (window, chunk) padded to exactly Q blocks of 128 edges -> fully
    static SPMD-uniform program (B = 4*Q blocks per window)
  * per 128-edge block: S_T[e, d] = (iota[e,d] == dstv[e]) * ew[e] via one
    DVE tensor_scalar (bf16 in/out -> 2x mode), then PE bf16 matmul
    psum[f, d] += G[e, f].T @ S_T[e, d] accumulated over the window's blocks
  * per window: PSUM -> bf16 SBUF accumulator column
  * epilogue: out2 = W @ acc (+bias) in 512-wide bf16 matmuls, DMA to HBM
  * host unpacks windows back to node order
"""

import os
import numpy as np
import ml_dtypes
from contextlib import ExitStack

import concourse.bass as bass
from concourse.bass import AP
import concourse.mybir as mybir
import concourse.tile as tile
from concourse import bacc
from concourse.bass_utils import run_bass_kernel_spmd

N_NODES = 100000
N_EDGES = 1600000
F = 128
P = 128
NC = 8
NPC = N_NODES // NC      # 12500 nodes per core
NCHUNKS = 4
CHUNK = N_NODES // NCHUNKS  # 25000 rows per src chunk (int16-addressable)

# Tunables
Q = int(os.environ.get("GCN_Q", "2"))        # blocks per (window, chunk)
NB = int(os.environ.get("GCN_NB", "25"))     # class-stream blocks per gather
G_BUFS = int(os.environ.get("GCN_GBUFS", "3"))
S_BUFS = int(os.environ.get("GCN_SBUFS", "6"))
PS_BUFS = int(os.environ.get("GCN_PSBUFS", "4"))
REPS = int(os.environ.get("GCN_REPS", "1"))  # in-NEFF repetitions (timing)
SKIP = os.environ.get("GCN_SKIP", "")        # '', 'gather', or 'compute'
GDT = os.environ.get("GCN_GDT", "bf16")      # gather dtype: 'bf16' | 'f32'
SP_PKT = os.environ.get("GCN_SP", "0") == "1"  # dma_gather single_packet
HALF = os.environ.get("GCN_HALF", "0") == "1"  # diag: half descs, 2x elem
STMODE = os.environ.get("GCN_STMODE", "batch")  # 'batch' | 'scalar'
# every ACTSPLIT-th window builds S_T on the Activation engine (0 = off)
ACTSPLIT = int(os.environ.get("GCN_ACTSPLIT", "3"))
# d-major strip layout: packed last dims -> DVE 2x mode eligible
ST2X = os.environ.get("GCN_ST2X", "0") == "1"
# window packing: 'greedy' contiguous scan | 'ff' first-fit node sets
PACK = os.environ.get("GCN_PACK", "greedy")

f32 = mybir.dt.float32
bf16 = mybir.dt.bfloat16
i16 = mybir.dt.int16

_PROGRAM_CACHE: dict = {}


def _dedup(edge_src, edge_dst, edge_weight, idx_keep):
    """Merge duplicate draws of the same edge into a weight multiplier."""
    cnt = np.bincount(np.asarray(idx_keep), minlength=N_EDGES)
    sel = np.nonzero(cnt)[0]
    src = np.asarray(edge_src)[sel].astype(np.int64)
    dst = np.asarray(edge_dst)[sel].astype(np.int64)
    ew = np.asarray(edge_weight)[sel].astype(np.float32) * cnt[sel].astype(
        np.float32
    )
    return src, dst, ew


def _preprocess(edge_src, edge_dst, edge_weight, idx_keep, q):
    """Shard kept (deduped) edges by dst, pack dst windows under per-chunk
    quotas, and emit the static device layout.

    Returns None if quotas are infeasible; caller bumps q.
    """
    src, dst, ew = _dedup(edge_src, edge_dst, edge_weight, idx_keep)
    order = np.argsort(dst, kind="stable")
    src, dst, ew = src[order], dst[order], ew[order]
    core_bounds = np.searchsorted(dst, np.arange(NC + 1) * NPC)

    QCAP = q * P
    B = NCHUNKS * q  # compute blocks per window
    percore = []
    for c in range(NC):
        lo, hi = int(core_bounds[c]), int(core_bounds[c + 1])
        dl = dst[lo:hi] - c * NPC
        ch = src[lo:hi] // CHUNK
        cnts = np.bincount(dl * NCHUNKS + ch, minlength=NPC * NCHUNKS).reshape(
            NPC, NCHUNKS
        )
        if PACK == "ff":
            if (cnts > QCAP).any():
                return None
            # first-fit decreasing over 4-dim chunk-count vectors
            order_n = np.argsort(-cnts.max(axis=1), kind="stable")
            wsums = np.zeros((0, NCHUNKS), np.int64)
            wcnt = np.zeros(0, np.int64)
            win_of = np.zeros(NPC, np.int64)
            pos_in = np.zeros(NPC, np.int64)
            wins = []
            for n in order_n:
                row = cnts[n]
                ok = (wcnt < P) & (wsums + row <= QCAP).all(axis=1)
                if ok.any():
                    # best-fit: fullest feasible window
                    load = wsums.sum(axis=1) + wcnt  # count as tiebreak
                    wi = int(np.argmax(np.where(ok, load, -1)))
                else:
                    wi = -1
                if wi < 0:
                    wi = len(wins)
                    wins.append([])
                    wsums = np.vstack([wsums, np.zeros((1, NCHUNKS),
                                                       np.int64)])
                    wcnt = np.append(wcnt, 0)
                win_of[n] = wi
                pos_in[n] = wcnt[wi]
                wins[wi].append(n)
                wsums[wi] += row
                wcnt[wi] += 1
            wins = [np.asarray(wn, np.int64) for wn in wins]
        else:
            wins = []
            n = 0
            cl = cnts.tolist()
            while n < NPC:
                s = n
                acc = [0, 0, 0, 0]
                while n < NPC and (n - s) < P:
                    row = cl[n]
                    if any(acc[m] + row[m] > QCAP for m in range(NCHUNKS)):
                        break
                    for m in range(NCHUNKS):
                        acc[m] += row[m]
                    n += 1
                if n == s:
                    return None
                wins.append((s, n))
        percore.append((lo, hi, dl, ch, cnts, wins))

    W_CAP = max(len(pc[5]) for pc in percore)
    W_CAP = -(-W_CAP // 4) * 4  # multiple of 4 -> epilogue chunks of 512
    NBLK = W_CAP * B             # compute blocks per core
    CSB = W_CAP * q              # class-stream blocks per chunk per core
    NIDX = CSB * P               # idxs per chunk stream

    idxbuf = np.zeros((NC, P, NCHUNKS * (NIDX // 16)), np.int16)
    dstv = np.zeros((NC, NBLK * P), np.float32)
    ewv = np.zeros((NC, NBLK * P), np.float32)
    metas = []
    for c, (lo, hi, dl, ch, cnts, wins) in enumerate(percore):
        ne = hi - lo
        key = dl * NCHUNKS + ch
        o2 = np.argsort(key, kind="stable")
        src_l = src[lo:hi][o2] - ch[o2] * CHUNK
        ew_l = ew[lo:hi][o2]
        dl_l = dl[o2]
        ch_l = ch[o2]
        S = np.zeros(NPC * NCHUNKS + 1, np.int64)
        np.cumsum(cnts.ravel(), out=S[1:])
        segstart = S[dl_l * NCHUNKS + ch_l]
        rank = np.arange(ne) - segstart
        if PACK == "ff":
            win_of = np.zeros(NPC, np.int64)
            colv = np.zeros(NPC, np.int64)
            off_seg = np.zeros((NPC, NCHUNKS), np.int64)
            for w, wn in enumerate(wins):
                win_of[wn] = w
                colv[wn] = np.arange(len(wn))
                pre = np.zeros((len(wn), NCHUNKS), np.int64)
                np.cumsum(cnts[wn][:-1], axis=0, out=pre[1:])
                off_seg[wn] = pre
            wj = win_of[dl_l]
            off_in_seg = off_seg[dl_l, ch_l]
            colw = colv[dl_l]
        else:
            Cn = np.zeros((NPC + 1, NCHUNKS), np.int64)
            np.cumsum(cnts, axis=0, out=Cn[1:])
            win_of = np.zeros(NPC, np.int64)
            wstart = np.zeros(NPC, np.int64)
            for w, (s, e) in enumerate(wins):
                win_of[s:e] = w
                wstart[s:e] = s
            wj = win_of[dl_l]
            swj = wstart[dl_l]
            off_in_seg = Cn[dl_l, ch_l] - Cn[swj, ch_l]
            colw = dl_l - swj
        slot = wj * (q * P) + off_in_seg + rank
        cols = slot // 16
        parts = slot % 16
        base_cols = ch_l * (NIDX // 16)
        flat16 = np.zeros((16, NCHUNKS * (NIDX // 16)), np.int16)
        flat16[parts, base_cols + cols] = src_l.astype(np.int16)
        idxbuf[c] = np.tile(flat16, (8, 1))
        cb = slot // P
        pp = slot % P
        qq = cb % q
        blk = wj * B + ch_l * q + qq
        dv = np.zeros((NBLK, P), np.float32)
        ev = np.zeros((NBLK, P), np.float32)
        dv[blk, pp] = colw.astype(np.float32)
        ev[blk, pp] = ew_l
        dstv[c] = dv.reshape(-1)
        ewv[c] = ev.reshape(-1)
        metas.append(wins)

    dstv = np.ascontiguousarray(dstv.reshape(NC, NBLK, P).transpose(0, 2, 1))
    ewv = np.ascontiguousarray(ewv.reshape(NC, NBLK, P).transpose(0, 2, 1))
    return idxbuf, dstv, ewv, metas, W_CAP, NBLK, CSB


def _build_program(W_CAP, q, NBLK, CSB):
    key = (W_CAP, q, NBLK, CSB, NB, G_BUFS, S_BUFS, PS_BUFS, REPS, SKIP, GDT,
           SP_PKT, HALF, STMODE, ACTSPLIT, ST2X)
    if key in _PROGRAM_CACHE:
        return _PROGRAM_CACHE[key]

    B = NCHUNKS * q
    NIDX = CSB * P
    IDXCOLS = NCHUNKS * (NIDX // 16)

    nc = bacc.Bacc(
        "TRN2",
        target_bir_lowering=False,
        debug=False,
        enable_asserts=False,
        num_devices=NC,
        num_swdge_queues=4,
    )
    gdt = bf16 if GDT == "bf16" else f32
    x_h = nc.dram_tensor("x16", [N_NODES, F], gdt, kind="ExternalInput")
    idx_d = nc.dram_tensor("idx", [P, IDXCOLS], i16, kind="ExternalInput").ap()
    sdt = bf16 if STMODE == "batch" else f32
    dstv_d = nc.dram_tensor("dstv", [P, NBLK], sdt, kind="ExternalInput").ap()
    ewv_d = nc.dram_tensor("ewv", [P, NBLK], sdt, kind="ExternalInput").ap()
    if ACTSPLIT:
        # fp32 per-partition scale/bias columns for the ACT-engine one-hot:
        # s_t = Relu(ew - ew*|iota - dst|)
        dstn_d = nc.dram_tensor("dstn", [P, NBLK], f32, kind="ExternalInput").ap()
        ewf_d = nc.dram_tensor("ewf", [P, NBLK], f32, kind="ExternalInput").ap()
        ewn_d = nc.dram_tensor("ewn", [P, NBLK], f32, kind="ExternalInput").ap()
    wt_d = nc.dram_tensor("wt", [P, P], bf16, kind="ExternalInput").ap()  # W.T
    bias_d = nc.dram_tensor("biasv", [P, 1], f32, kind="ExternalInput").ap()
    out_d = nc.dram_tensor("out", [P, W_CAP * P], f32, kind="ExternalOutput").ap()

    iota_np = np.broadcast_to(
        np.arange(P, dtype=np.float32), (P, P)
    ).astype(ml_dtypes.bfloat16)
    iota_d = nc.inline_tensor(iota_np, "iota").ap()
    if ST2X:
        # iota repeated B-wise: col c = d*B + bl -> value d
        iotar_np = np.broadcast_to(
            np.repeat(np.arange(P, dtype=np.float32), B), (P, B * P)
        ).astype(ml_dtypes.bfloat16)
        iotar_d = nc.inline_tensor(iotar_np, "iotar").ap()

    NOCHUNK = (W_CAP * P) // 512
    NGC = (CSB + NB - 1) // NB  # gather calls per chunk

    with tile.TileContext(nc) as tc, ExitStack() as ctx:
        const = ctx.enter_context(tc.tile_pool(name="const", bufs=1))
        gpools = [
            ctx.enter_context(tc.tile_pool(name=f"g{m}", bufs=G_BUFS))
            for m in range(NCHUNKS)
        ]
        spool = ctx.enter_context(tc.tile_pool(name="s", bufs=S_BUFS))
        pspool = ctx.enter_context(tc.tile_pool(name="ps", bufs=PS_BUFS, space="PSUM"))
        ps2pool = ctx.enter_context(tc.tile_pool(name="ps2", bufs=2, space="PSUM"))
        stpool = ctx.enter_context(tc.tile_pool(name="st", bufs=2))

        iota_sb = const.tile([P, P], bf16)
        nc.sync.dma_start(out=iota_sb[:], in_=iota_d[:])
        if ST2X:
            iotar_sb = const.tile([P, B * P], bf16)
            nc.sync.dma_start(out=iotar_sb[:], in_=iotar_d[:])
        wt_sb = const.tile([P, P], bf16)
        nc.sync.dma_start(out=wt_sb[:], in_=wt_d[:])
        bias_sb = const.tile([P, 1], f32)
        nc.sync.dma_start(out=bias_sb[:], in_=bias_d[:])
        idx_sb = const.tile([P, IDXCOLS], i16)
        for m in range(NCHUNKS):  # per-chunk loads: first gathers start sooner
            nc.sync.dma_start(
                out=idx_sb[:, m * (NIDX // 16):(m + 1) * (NIDX // 16)],
                in_=idx_d[:, m * (NIDX // 16):(m + 1) * (NIDX // 16)],
            )
        dstv_sb = const.tile([P, NBLK], sdt)
        nc.sync.dma_start(out=dstv_sb[:], in_=dstv_d[:])
        ewv_sb = const.tile([P, NBLK], sdt)
        nc.sync.dma_start(out=ewv_sb[:], in_=ewv_d[:])
        if ACTSPLIT:
            dstn_sb = const.tile([P, NBLK], f32)
            nc.sync.dma_start(out=dstn_sb[:], in_=dstn_d[:])
            ewf_sb = const.tile([P, NBLK], f32)
            nc.sync.dma_start(out=ewf_sb[:], in_=ewf_d[:])
            ewn_sb = const.tile([P, NBLK], f32)
            nc.sync.dma_start(out=ewn_sb[:], in_=ewn_d[:])
        acc = const.tile([P, W_CAP * P], bf16)

        g_tiles = {}

        def body():
            g_tiles.clear()
            if SKIP == "compute":
                for t in range(NGC):  # t-major: keep all 4 queues busy
                    for m in range(NCHUNKS):
                        ensure_gather(m, t)
                return
            for w in range(W_CAP):
                ps = pspool.tile([P, P], f32, space="PSUM")
                act_win = ACTSPLIT and (w % ACTSPLIT == ACTSPLIT - 1)
                if act_win:
                    pass  # per-block ACT build below
                elif STMODE == "batch":
                    # one S_T strip for all B blocks of the window:
                    # tmp = (iota bcast) == (dstv bcast); s_t = tmp * ew
                    s_t = spool.tile([P, B * P], bf16)
                    tmp = spool.tile([P, B * P], bf16)
                    if ST2X:
                        # d-major cols (c = d*B + bl): all last dims packed
                        in0 = iotar_sb[:]
                        dstv_b = AP(
                            dstv_sb.tensor,
                            dstv_sb[:, w * B:(w + 1) * B].offset,
                            [dstv_sb[:].ap[0], (0, P), (1, B)],
                        )
                        ewv_b = AP(
                            ewv_sb.tensor,
                            ewv_sb[:, w * B:(w + 1) * B].offset,
                            [ewv_sb[:].ap[0], (0, P), (1, B)],
                        )
                    else:
                        in0 = AP(
                            iota_sb.tensor,
                            iota_sb[:].offset,
                            [iota_sb[:].ap[0], (0, B), (1, P)],
                        )
                        dstv_b = AP(
                            dstv_sb.tensor,
                            dstv_sb[:, w * B:(w + 1) * B].offset,
                            [dstv_sb[:].ap[0], (1, B), (0, P)],
                        )
                        ewv_b = AP(
                            ewv_sb.tensor,
                            ewv_sb[:, w * B:(w + 1) * B].offset,
                            [ewv_sb[:].ap[0], (1, B), (0, P)],
                        )
                    nc.vector.tensor_tensor(
                        out=tmp[:], in0=in0, in1=dstv_b,
                        op=mybir.AluOpType.is_equal,
                    )
                    nc.vector.tensor_tensor(
                        out=s_t[:], in0=tmp[:], in1=ewv_b,
                        op=mybir.AluOpType.mult,
                    )
                for m in range(NCHUNKS):
                    for qq in range(q):
                        blk = w * B + m * q + qq
                        bl = m * q + qq  # block index within window
                        cb = w * q + qq
                        t, col = divmod(cb, NB)
                        g = ensure_gather(m, t)
                        if act_win:
                            u = spool.tile([P, P], bf16)
                            nc.scalar.activation(
                                out=u[:], in_=iota_sb[:],
                                func=mybir.ActivationFunctionType.Abs,
                                bias=dstn_sb[:, blk:blk + 1], scale=1.0,
                            )
                            s1 = spool.tile([P, P], bf16)
                            nc.scalar.activation(
                                out=s1[:], in_=u[:],
                                func=mybir.ActivationFunctionType.Relu,
                                bias=ewf_sb[:, blk:blk + 1],
                                scale=ewn_sb[:, blk:blk + 1],
                            )
                            rhs = s1[:]
                        elif STMODE == "batch":
                            if ST2X:
                                # column d of block bl lives at c = d*B + bl
                                rhs = AP(
                                    s_t.tensor,
                                    s_t[:].offset + bl,
                                    [s_t[:].ap[0], (B, P)],
                                )
                            else:
                                rhs = s_t[:, bl * P:(bl + 1) * P]
                        else:
                            s1 = spool.tile([P, P], bf16)
                            nc.vector.tensor_scalar(
                                out=s1[:],
                                in0=iota_sb[:],
                                scalar1=dstv_sb[:, blk:blk + 1],
                                scalar2=ewv_sb[:, blk:blk + 1],
                                op0=mybir.AluOpType.is_equal,
                                op1=mybir.AluOpType.mult,
                            )
                            rhs = s1[:]
                        first = m == 0 and qq == 0
                        last = m == NCHUNKS - 1 and qq == q - 1
                        nc.tensor.matmul(
                            out=ps[:],
                            lhsT=g[:, col, :],
                            rhs=rhs,
                            start=first,
                            stop=last,
                        )
                if act_win:  # keep ACT free on its windows
                    nc.vector.tensor_copy(
                        out=acc[:, w * P:(w + 1) * P], in_=ps[:]
                    )
                else:
                    nc.scalar.copy(out=acc[:, w * P:(w + 1) * P], in_=ps[:])
                if (w + 1) % 4 == 0:  # stream the epilogue as acc fills
                    epi_chunk((w + 1) // 4 - 1)

        def epilogue():
            for cix in range(NOCHUNK):
                epi_chunk(cix)

        def epi_chunk(cix):
                ps2 = ps2pool.tile([P, 512], f32, space="PSUM")
                nc.tensor.matmul(
                    out=ps2[:],
                    lhsT=wt_sb[:],
                    rhs=acc[:, cix * 512:(cix + 1) * 512],
                    start=True,
                    stop=True,
                )
                st = stpool.tile([P, 512], f32)
                nc.scalar.add(out=st[:], in_=ps2[:], add=bias_sb[:, 0:1])
                nc.sync.dma_start(out=out_d[:, cix * 512:(cix + 1) * 512], in_=st[:])

        def ensure_gather(m, t):
            if (m, t) in g_tiles:
                return g_tiles[(m, t)]
            nb = min(NB, CSB - t * NB)
            n_idx = nb * P
            g = gpools[m].tile([P, NB, F], gdt)
            if SKIP == "gather":
                # sequential-stream stand-in write: keeps the tile written
                # (framework requirement) at streaming DMA cost, no descgen
                nc.sync.dma_start(
                    out=g[:, :nb, :],
                    in_=AP(x_h, m * CHUNK * F,
                           [(F, P), (F * P, nb), (1, F)]),
                )
                g_tiles[(m, t)] = g
                return g
            if HALF:
                # diagnostic only (wrong data): same bytes, half descriptors
                g2 = gpools[m].tile([P, NB // 2, 2 * F], gdt)
                g_tiles[(m, t)] = g2
                nc.gpsimd.dma_gather(
                    out_ap=g2[:, :nb // 2, :],
                    in_ap=AP(x_h, 0, [(2 * F, CHUNK), (1, 2 * F)]),
                    idxs_ap=idx_sb[
                        :, m * (NIDX // 16) + t * NB * 8:
                           m * (NIDX // 16) + t * NB * 8 + n_idx // 32
                    ],
                    num_idxs=n_idx // 2,
                    num_idxs_reg=n_idx // 2,
                    elem_size=2 * F,
                    single_packet=SP_PKT,
                    queue_num=m,
                )
                return g2
            else:
                nc.gpsimd.dma_gather(
                    out_ap=g[:, :nb, :],
                    in_ap=AP(x_h, m * CHUNK * F, [(F, CHUNK), (1, F)]),
                    idxs_ap=idx_sb[
                        :, m * (NIDX // 16) + t * NB * 8:
                           m * (NIDX // 16) + t * NB * 8 + n_idx // 16
                    ],
                    num_idxs=n_idx,
                    num_idxs_reg=n_idx,
                    elem_size=F,
                    single_packet=SP_PKT,
                    queue_num=m,
                )
            g_tiles[(m, t)] = g
            return g

        if REPS > 1:
            with tc.For_i(0, REPS, 1):
                body()
        else:
            body()

    nc.compile()
    _PROGRAM_CACHE[key] = nc
    return nc


def _prepare(x, W, bias, edge_src, edge_dst, edge_weight, idx_keep):
    q = Q
    while True:
        pre = _preprocess(edge_src, edge_dst, edge_weight, idx_keep, q)
        if pre is not None:
            break
        q += 1
    idxbuf, dstv, ewv, metas, W_CAP, NBLK, CSB = pre
    nc = _build_program(W_CAP, q, NBLK, CSB)

    xf = np.ascontiguousarray(np.asarray(x, dtype=np.float32))
    x16 = xf.astype(ml_dtypes.bfloat16) if GDT == "bf16" else xf
    wt = np.ascontiguousarray(
        np.asarray(W, dtype=np.float32).T
    ).astype(ml_dtypes.bfloat16)
    biasv = np.ascontiguousarray(np.asarray(bias, dtype=np.float32).reshape(P, 1))
    dstv32, ewv32 = dstv, ewv
    if STMODE == "batch":
        dstv = dstv.astype(ml_dtypes.bfloat16)
        ewv = ewv.astype(ml_dtypes.bfloat16)
    in_maps = [
        {
            "x16": x16,
            "idx": idxbuf[c],
            "dstv": dstv[c],
            "ewv": ewv[c],
            "wt": wt,
            "biasv": biasv,
        }
        for c in range(NC)
    ]
    if ACTSPLIT:
        for c in range(NC):
            in_maps[c]["dstn"] = -dstv32[c]
            in_maps[c]["ewf"] = ewv32[c]
            in_maps[c]["ewn"] = -ewv32[c]
    return nc, in_maps, metas


def _unpack(results, metas):
    out = np.empty((N_NODES, F), np.float32)
    for c in range(NC):
        o = results[c]["out"]  # [P, W_CAP*P], rows = out features
        base = c * NPC
        for w, win in enumerate(metas[c]):
            if isinstance(win, tuple):
                s, e = win
                out[base + s:base + e, :] = o[:, w * P:w * P + (e - s)].T
            else:  # node array in placement (column) order
                out[base + win, :] = o[:, w * P:w * P + len(win)].T
    return out


def kernel(x, W, bias, edge_src, edge_dst, edge_weight, idx_keep):
    nc, in_maps, metas = _prepare(
        x, W, bias, edge_src, edge_dst, edge_weight, idx_keep
    )
    res = run_bass_kernel_spmd(nc, in_maps, list(range(NC)))
    return _unpack(res.results, metas)


# --- helpers for test.py (not used by the grading harness) ---------------

def run_traced(x, W, bias, edge_src, edge_dst, edge_weight, idx_keep):
    nc, in_maps, metas = _prepare(
        x, W, bias, edge_src, edge_dst, edge_weight, idx_keep
    )
    res = run_bass_kernel_spmd(nc, in_maps, list(range(NC)), trace=True)
    return _unpack(res.results, metas), res


def run_sim(x, W, bias, edge_src, edge_dst, edge_weight, idx_keep, cores=(0,)):
    from concourse.bass_interp import CoreSim

    nc, in_maps, metas = _prepare(
        x, W, bias, edge_src, edge_dst, edge_weight, idx_keep
    )
    results = []
    for c in cores:
        sim = CoreSim(nc)
        for k, v in in_maps[c].items():
            sim.tensor(k)[:] = v
        sim.simulate()
        results.append({"out": sim.tensor("out").copy()})
    return results, metas, in_maps


# revision 63
# speedup vs baseline: 4.0244x; 1.2336x over previous
"""GCNConvRnd kernel for 8 Trainium2 NeuronCores (Bass/Tile) — v2 (bf16).

out = segment_sum((x @ W.T)[src[keep]] * ew[keep], dst[keep], N) + bias

Strategy (dst-sharded, W applied after aggregation):
  * kept edges deduped on host (idx_keep samples WITH replacement: ~31%
    duplicates merge into edge-weight multiplicity)
  * x cast to bf16 on host and shipped as the gather source: halves gather
    bytes and makes every matmul a 1-cycle/row bf16 matmul
  * nodes / output sharded 12500 per core; kept edges partitioned by dst
  * each core gathers x16[src] rows with GPSIMD dma_gather (int16 indices,
    4 chunks of 25000 rows so local index fits int16)
  * edges sorted by dst, packed into windows of <=128 consecutive dst nodes,
    each (window, chunk) padded to exactly Q blocks of 128 edges -> fully
    static SPMD-uniform program (B = 4*Q blocks per window)
  * per 128-edge block: S_T[e, d] = (iota[e,d] == dstv[e]) * ew[e] via one
    DVE tensor_scalar (bf16 in/out -> 2x mode), then PE bf16 matmul
    psum[f, d] += G[e, f].T @ S_T[e, d] accumulated over the window's blocks
  * per window: PSUM -> bf16 SBUF accumulator column
  * epilogue: out2 = W @ acc (+bias) in 512-wide bf16 matmuls, DMA to HBM
  * host unpacks windows back to node order
"""

import os
import numpy as np
import ml_dtypes
from contextlib import ExitStack

import concourse.bass as bass
from concourse.bass import AP
import concourse.mybir as mybir
import concourse.tile as tile
from concourse import bacc
from concourse.bass_utils import run_bass_kernel_spmd

N_NODES = 100000
N_EDGES = 1600000
F = 128
P = 128
NC = 8
NPC = N_NODES // NC      # 12500 nodes per core
NCHUNKS = 4
CHUNK = N_NODES // NCHUNKS  # 25000 rows per src chunk (int16-addressable)

# Tunables
Q = int(os.environ.get("GCN_Q", "2"))        # blocks per (window, chunk)
NB = int(os.environ.get("GCN_NB", "25"))     # class-stream blocks per gather
G_BUFS = int(os.environ.get("GCN_GBUFS", "4"))
S_BUFS = int(os.environ.get("GCN_SBUFS", "8"))
PS_BUFS = int(os.environ.get("GCN_PSBUFS", "6"))
REPS = int(os.environ.get("GCN_REPS", "1"))  # in-NEFF repetitions (timing)
SKIP = os.environ.get("GCN_SKIP", "")        # '', 'gather', or 'compute'
GDT = os.environ.get("GCN_GDT", "bf16")      # gather dtype: 'bf16' | 'f32'
SP_PKT = os.environ.get("GCN_SP", "0") == "1"  # dma_gather single_packet
HALF = os.environ.get("GCN_HALF", "0") == "1"  # diag: half descs, 2x elem
STMODE = os.environ.get("GCN_STMODE", "batch")  # 'batch' | 'scalar'
# every ACTSPLIT-th window builds S_T on the Activation engine (0 = off)
ACTSPLIT = int(os.environ.get("GCN_ACTSPLIT", "3"))
# d-major strip layout: packed last dims -> DVE 2x mode eligible
ST2X = os.environ.get("GCN_ST2X", "0") == "1"
# window packing: 'greedy' contiguous scan | 'ff' first-fit node sets
PACK = os.environ.get("GCN_PACK", "greedy")
# size of the first gather call per queue (0 = uniform NB); small first
# call lets compute start sooner
FIRSTNB = int(os.environ.get("GCN_FIRSTNB", "0"))

f32 = mybir.dt.float32
bf16 = mybir.dt.bfloat16
i16 = mybir.dt.int16

_PROGRAM_CACHE: dict = {}


def _dedup(edge_src, edge_dst, edge_weight, idx_keep):
    """Merge duplicate draws of the same edge into a weight multiplier."""
    cnt = np.bincount(np.asarray(idx_keep), minlength=N_EDGES)
    sel = np.nonzero(cnt)[0]
    src = np.asarray(edge_src)[sel].astype(np.int64)
    dst = np.asarray(edge_dst)[sel].astype(np.int64)
    ew = np.asarray(edge_weight)[sel].astype(np.float32) * cnt[sel].astype(
        np.float32
    )
    return src, dst, ew


def _preprocess(edge_src, edge_dst, edge_weight, idx_keep, q):
    """Shard kept (deduped) edges by dst, pack dst windows under per-chunk
    quotas, and emit the static device layout.

    Returns None if quotas are infeasible; caller bumps q.
    """
    src, dst, ew = _dedup(edge_src, edge_dst, edge_weight, idx_keep)
    order = np.argsort(dst, kind="stable")
    src, dst, ew = src[order], dst[order], ew[order]
    core_bounds = np.searchsorted(dst, np.arange(NC + 1) * NPC)

    QCAP = q * P
    B = NCHUNKS * q  # compute blocks per window
    percore = []
    for c in range(NC):
        lo, hi = int(core_bounds[c]), int(core_bounds[c + 1])
        dl = dst[lo:hi] - c * NPC
        ch = src[lo:hi] // CHUNK
        cnts = np.bincount(dl * NCHUNKS + ch, minlength=NPC * NCHUNKS).reshape(
            NPC, NCHUNKS
        )
        if PACK == "ff":
            if (cnts > QCAP).any():
                return None
            # first-fit decreasing over 4-dim chunk-count vectors
            order_n = np.argsort(-cnts.max(axis=1), kind="stable")
            wsums = np.zeros((0, NCHUNKS), np.int64)
            wcnt = np.zeros(0, np.int64)
            win_of = np.zeros(NPC, np.int64)
            pos_in = np.zeros(NPC, np.int64)
            wins = []
            for n in order_n:
                row = cnts[n]
                ok = (wcnt < P) & (wsums + row <= QCAP).all(axis=1)
                if ok.any():
                    # best-fit: fullest feasible window
                    load = wsums.sum(axis=1) + wcnt  # count as tiebreak
                    wi = int(np.argmax(np.where(ok, load, -1)))
                else:
                    wi = -1
                if wi < 0:
                    wi = len(wins)
                    wins.append([])
                    wsums = np.vstack([wsums, np.zeros((1, NCHUNKS),
                                                       np.int64)])
                    wcnt = np.append(wcnt, 0)
                win_of[n] = wi
                pos_in[n] = wcnt[wi]
                wins[wi].append(n)
                wsums[wi] += row
                wcnt[wi] += 1
            wins = [np.asarray(wn, np.int64) for wn in wins]
        else:
            wins = []
            n = 0
            cl = cnts.tolist()
            while n < NPC:
                s = n
                acc = [0, 0, 0, 0]
                while n < NPC and (n - s) < P:
                    row = cl[n]
                    if any(acc[m] + row[m] > QCAP for m in range(NCHUNKS)):
                        break
                    for m in range(NCHUNKS):
                        acc[m] += row[m]
                    n += 1
                if n == s:
                    return None
                wins.append((s, n))
        percore.append((lo, hi, dl, ch, cnts, wins))

    W_CAP = max(len(pc[5]) for pc in percore)
    W_CAP = -(-W_CAP // 4) * 4  # multiple of 4 -> epilogue chunks of 512
    NBLK = W_CAP * B             # compute blocks per core
    CSB = W_CAP * q              # class-stream blocks per chunk per core
    NIDX = CSB * P               # idxs per chunk stream

    idxbuf = np.zeros((NC, P, NCHUNKS * (NIDX // 16)), np.int16)
    dstv = np.zeros((NC, NBLK * P), np.float32)
    ewv = np.zeros((NC, NBLK * P), np.float32)
    metas = []
    for c, (lo, hi, dl, ch, cnts, wins) in enumerate(percore):
        ne = hi - lo
        key = dl * NCHUNKS + ch
        o2 = np.argsort(key, kind="stable")
        src_l = src[lo:hi][o2] - ch[o2] * CHUNK
        ew_l = ew[lo:hi][o2]
        dl_l = dl[o2]
        ch_l = ch[o2]
        S = np.zeros(NPC * NCHUNKS + 1, np.int64)
        np.cumsum(cnts.ravel(), out=S[1:])
        segstart = S[dl_l * NCHUNKS + ch_l]
        rank = np.arange(ne) - segstart
        if PACK == "ff":
            win_of = np.zeros(NPC, np.int64)
            colv = np.zeros(NPC, np.int64)
            off_seg = np.zeros((NPC, NCHUNKS), np.int64)
            for w, wn in enumerate(wins):
                win_of[wn] = w
                colv[wn] = np.arange(len(wn))
                pre = np.zeros((len(wn), NCHUNKS), np.int64)
                np.cumsum(cnts[wn][:-1], axis=0, out=pre[1:])
                off_seg[wn] = pre
            wj = win_of[dl_l]
            off_in_seg = off_seg[dl_l, ch_l]
            colw = colv[dl_l]
        else:
            Cn = np.zeros((NPC + 1, NCHUNKS), np.int64)
            np.cumsum(cnts, axis=0, out=Cn[1:])
            win_of = np.zeros(NPC, np.int64)
            wstart = np.zeros(NPC, np.int64)
            for w, (s, e) in enumerate(wins):
                win_of[s:e] = w
                wstart[s:e] = s
            wj = win_of[dl_l]
            swj = wstart[dl_l]
            off_in_seg = Cn[dl_l, ch_l] - Cn[swj, ch_l]
            colw = dl_l - swj
        slot = wj * (q * P) + off_in_seg + rank
        cols = slot // 16
        parts = slot % 16
        base_cols = ch_l * (NIDX // 16)
        flat16 = np.zeros((16, NCHUNKS * (NIDX // 16)), np.int16)
        flat16[parts, base_cols + cols] = src_l.astype(np.int16)
        idxbuf[c] = np.tile(flat16, (8, 1))
        cb = slot // P
        pp = slot % P
        qq = cb % q
        blk = wj * B + ch_l * q + qq
        dv = np.zeros((NBLK, P), np.float32)
        ev = np.zeros((NBLK, P), np.float32)
        dv[blk, pp] = colw.astype(np.float32)
        ev[blk, pp] = ew_l
        dstv[c] = dv.reshape(-1)
        ewv[c] = ev.reshape(-1)
        metas.append(wins)

    dstv = np.ascontiguousarray(dstv.reshape(NC, NBLK, P).transpose(0, 2, 1))
    ewv = np.ascontiguousarray(ewv.reshape(NC, NBLK, P).transpose(0, 2, 1))
    return idxbuf, dstv, ewv, metas, W_CAP, NBLK, CSB


def _build_program(W_CAP, q, NBLK, CSB):
    key = (W_CAP, q, NBLK, CSB, NB, G_BUFS, S_BUFS, PS_BUFS, REPS, SKIP, GDT,
           SP_PKT, HALF, STMODE, ACTSPLIT, ST2X, FIRSTNB)
    if key in _PROGRAM_CACHE:
        return _PROGRAM_CACHE[key]

    B = NCHUNKS * q
    NIDX = CSB * P
    IDXCOLS = NCHUNKS * (NIDX // 16)

    nc = bacc.Bacc(
        "TRN2",
        target_bir_lowering=False,
        debug=False,
        enable_asserts=False,
        num_devices=NC,
        num_swdge_queues=4,
    )
    gdt = bf16 if GDT == "bf16" else f32
    x_h = nc.dram_tensor("x16", [N_NODES, F], gdt, kind="ExternalInput")
    idx_d = nc.dram_tensor("idx", [P, IDXCOLS], i16, kind="ExternalInput").ap()
    sdt = bf16 if STMODE == "batch" else f32
    dstv_d = nc.dram_tensor("dstv", [P, NBLK], sdt, kind="ExternalInput").ap()
    ewv_d = nc.dram_tensor("ewv", [P, NBLK], sdt, kind="ExternalInput").ap()
    if ACTSPLIT:
        # fp32 per-partition scale/bias columns for the ACT-engine one-hot:
        # s_t = Relu(ew - ew*|iota - dst|)
        dstn_d = nc.dram_tensor("dstn", [P, NBLK], f32, kind="ExternalInput").ap()
        ewf_d = nc.dram_tensor("ewf", [P, NBLK], f32, kind="ExternalInput").ap()
        ewn_d = nc.dram_tensor("ewn", [P, NBLK], f32, kind="ExternalInput").ap()
    wt_d = nc.dram_tensor("wt", [P, P], bf16, kind="ExternalInput").ap()  # W.T
    bias_d = nc.dram_tensor("biasv", [P, 1], f32, kind="ExternalInput").ap()
    out_d = nc.dram_tensor("out", [P, W_CAP * P], f32, kind="ExternalOutput").ap()

    iota_np = np.broadcast_to(
        np.arange(P, dtype=np.float32), (P, P)
    ).astype(ml_dtypes.bfloat16)
    iota_d = nc.inline_tensor(iota_np, "iota").ap()
    if ST2X:
        # iota repeated B-wise: col c = d*B + bl -> value d
        iotar_np = np.broadcast_to(
            np.repeat(np.arange(P, dtype=np.float32), B), (P, B * P)
        ).astype(ml_dtypes.bfloat16)
        iotar_d = nc.inline_tensor(iotar_np, "iotar").ap()

    NOCHUNK = (W_CAP * P) // 512
    # gather call sizes per chunk stream (small first call -> early compute)
    SIZES = []
    left = CSB
    if FIRSTNB and FIRSTNB < min(NB, CSB):
        SIZES.append(FIRSTNB)
        left -= FIRSTNB
    while left > 0:
        s = min(NB, left)
        SIZES.append(s)
        left -= s
    OFFS = [0]
    for s in SIZES:
        OFFS.append(OFFS[-1] + s)
    call_of = []
    col_of = []
    for t, s in enumerate(SIZES):
        call_of += [t] * s
        col_of += list(range(s))
    NGC = len(SIZES)  # gather calls per chunk

    with tile.TileContext(nc) as tc, ExitStack() as ctx:
        const = ctx.enter_context(tc.tile_pool(name="const", bufs=1))
        gpools = [
            ctx.enter_context(tc.tile_pool(name=f"g{m}", bufs=G_BUFS))
            for m in range(NCHUNKS)
        ]
        spool = ctx.enter_context(tc.tile_pool(name="s", bufs=S_BUFS))
        pspool = ctx.enter_context(tc.tile_pool(name="ps", bufs=PS_BUFS, space="PSUM"))
        ps2pool = ctx.enter_context(tc.tile_pool(name="ps2", bufs=2, space="PSUM"))
        stpool = ctx.enter_context(tc.tile_pool(name="st", bufs=2))

        iota_sb = const.tile([P, P], bf16)
        nc.sync.dma_start(out=iota_sb[:], in_=iota_d[:])
        if ST2X:
            iotar_sb = const.tile([P, B * P], bf16)
            nc.sync.dma_start(out=iotar_sb[:], in_=iotar_d[:])
        wt_sb = const.tile([P, P], bf16)
        nc.sync.dma_start(out=wt_sb[:], in_=wt_d[:])
        bias_sb = const.tile([P, 1], f32)
        nc.sync.dma_start(out=bias_sb[:], in_=bias_d[:])
        idx_sb = const.tile([P, IDXCOLS], i16)
        for m in range(NCHUNKS):  # per-chunk loads: first gathers start sooner
            nc.sync.dma_start(
                out=idx_sb[:, m * (NIDX // 16):(m + 1) * (NIDX // 16)],
                in_=idx_d[:, m * (NIDX // 16):(m + 1) * (NIDX // 16)],
            )
        dstv_sb = const.tile([P, NBLK], sdt)
        nc.sync.dma_start(out=dstv_sb[:], in_=dstv_d[:])
        ewv_sb = const.tile([P, NBLK], sdt)
        nc.sync.dma_start(out=ewv_sb[:], in_=ewv_d[:])
        if ACTSPLIT:
            dstn_sb = const.tile([P, NBLK], f32)
            nc.sync.dma_start(out=dstn_sb[:], in_=dstn_d[:])
            ewf_sb = const.tile([P, NBLK], f32)
            nc.sync.dma_start(out=ewf_sb[:], in_=ewf_d[:])
            ewn_sb = const.tile([P, NBLK], f32)
            nc.sync.dma_start(out=ewn_sb[:], in_=ewn_d[:])
        acc = const.tile([P, W_CAP * P], bf16)

        g_tiles = {}

        def body():
            g_tiles.clear()
            if SKIP == "compute":
                for t in range(NGC):  # t-major: keep all 4 queues busy
                    for m in range(NCHUNKS):
                        ensure_gather(m, t)
                return
            for w in range(W_CAP):
                ps = pspool.tile([P, P], f32, space="PSUM")
                act_win = ACTSPLIT and (w % ACTSPLIT == ACTSPLIT - 1)
                if act_win:
                    pass  # per-block ACT build below
                elif STMODE == "batch":
                    # one S_T strip for all B blocks of the window:
                    # tmp = (iota bcast) == (dstv bcast); s_t = tmp * ew
                    s_t = spool.tile([P, B * P], bf16)
                    tmp = spool.tile([P, B * P], bf16)
                    if ST2X:
                        # d-major cols (c = d*B + bl): all last dims packed
                        in0 = iotar_sb[:]
                        dstv_b = AP(
                            dstv_sb.tensor,
                            dstv_sb[:, w * B:(w + 1) * B].offset,
                            [dstv_sb[:].ap[0], (0, P), (1, B)],
                        )
                        ewv_b = AP(
                            ewv_sb.tensor,
                            ewv_sb[:, w * B:(w + 1) * B].offset,
                            [ewv_sb[:].ap[0], (0, P), (1, B)],
                        )
                    else:
                        in0 = AP(
                            iota_sb.tensor,
                            iota_sb[:].offset,
                            [iota_sb[:].ap[0], (0, B), (1, P)],
                        )
                        dstv_b = AP(
                            dstv_sb.tensor,
                            dstv_sb[:, w * B:(w + 1) * B].offset,
                            [dstv_sb[:].ap[0], (1, B), (0, P)],
                        )
                        ewv_b = AP(
                            ewv_sb.tensor,
                            ewv_sb[:, w * B:(w + 1) * B].offset,
                            [ewv_sb[:].ap[0], (1, B), (0, P)],
                        )
                    nc.vector.tensor_tensor(
                        out=tmp[:], in0=in0, in1=dstv_b,
                        op=mybir.AluOpType.is_equal,
                    )
                    nc.vector.tensor_tensor(
                        out=s_t[:], in0=tmp[:], in1=ewv_b,
                        op=mybir.AluOpType.mult,
                    )
                for m in range(NCHUNKS):
                    for qq in range(q):
                        blk = w * B + m * q + qq
                        bl = m * q + qq  # block index within window
                        cb = w * q + qq
                        t, col = call_of[cb], col_of[cb]
                        g = ensure_gather(m, t)
                        if act_win:
                            u = spool.tile([P, P], bf16)
                            nc.scalar.activation(
                                out=u[:], in_=iota_sb[:],
                                func=mybir.ActivationFunctionType.Abs,
                                bias=dstn_sb[:, blk:blk + 1], scale=1.0,
                            )
                            s1 = spool.tile([P, P], bf16)
                            nc.scalar.activation(
                                out=s1[:], in_=u[:],
                                func=mybir.ActivationFunctionType.Relu,
                                bias=ewf_sb[:, blk:blk + 1],
                                scale=ewn_sb[:, blk:blk + 1],
                            )
                            rhs = s1[:]
                        elif STMODE == "batch":
                            if ST2X:
                                # column d of block bl lives at c = d*B + bl
                                rhs = AP(
                                    s_t.tensor,
                                    s_t[:].offset + bl,
                                    [s_t[:].ap[0], (B, P)],
                                )
                            else:
                                rhs = s_t[:, bl * P:(bl + 1) * P]
                        else:
                            s1 = spool.tile([P, P], bf16)
                            nc.vector.tensor_scalar(
                                out=s1[:],
                                in0=iota_sb[:],
                                scalar1=dstv_sb[:, blk:blk + 1],
                                scalar2=ewv_sb[:, blk:blk + 1],
                                op0=mybir.AluOpType.is_equal,
                                op1=mybir.AluOpType.mult,
                            )
                            rhs = s1[:]
                        first = m == 0 and qq == 0
                        last = m == NCHUNKS - 1 and qq == q - 1
                        nc.tensor.matmul(
                            out=ps[:],
                            lhsT=g[:, col, :],
                            rhs=rhs,
                            start=first,
                            stop=last,
                        )
                if act_win:  # keep ACT free on its windows
                    nc.vector.tensor_copy(
                        out=acc[:, w * P:(w + 1) * P], in_=ps[:]
                    )
                else:
                    nc.scalar.copy(out=acc[:, w * P:(w + 1) * P], in_=ps[:])
                if (w + 1) % 4 == 0:  # stream the epilogue as acc fills
                    epi_chunk((w + 1) // 4 - 1)

        def epilogue():
            for cix in range(NOCHUNK):
                epi_chunk(cix)

        def epi_chunk(cix):
                ps2 = ps2pool.tile([P, 512], f32, space="PSUM")
                nc.tensor.matmul(
                    out=ps2[:],
                    lhsT=wt_sb[:],
                    rhs=acc[:, cix * 512:(cix + 1) * 512],
                    start=True,
                    stop=True,
                )
                st = stpool.tile([P, 512], f32)
                nc.scalar.add(out=st[:], in_=ps2[:], add=bias_sb[:, 0:1])
                nc.sync.dma_start(out=out_d[:, cix * 512:(cix + 1) * 512], in_=st[:])

        def ensure_gather(m, t):
            if (m, t) in g_tiles:
                return g_tiles[(m, t)]
            nb = SIZES[t]
            n_idx = nb * P
            g = gpools[m].tile([P, nb, F], gdt)
            if SKIP == "gather":
                # sequential-stream stand-in write: keeps the tile written
                # (framework requirement) at streaming DMA cost, no descgen
                nc.sync.dma_start(
                    out=g[:, :nb, :],
                    in_=AP(x_h, m * CHUNK * F,
                           [(F, P), (F * P, nb), (1, F)]),
                )
                g_tiles[(m, t)] = g
                return g
            if HALF:
                # diagnostic only (wrong data): same bytes, half descriptors
                g2 = gpools[m].tile([P, max(nb // 2, 1), 2 * F], gdt)
                g_tiles[(m, t)] = g2
                nc.gpsimd.dma_gather(
                    out_ap=g2[:, :nb // 2, :],
                    in_ap=AP(x_h, 0, [(2 * F, CHUNK), (1, 2 * F)]),
                    idxs_ap=idx_sb[
                        :, m * (NIDX // 16) + OFFS[t] * 8:
                           m * (NIDX // 16) + OFFS[t] * 8 + n_idx // 32
                    ],
                    num_idxs=n_idx // 2,
                    num_idxs_reg=n_idx // 2,
                    elem_size=2 * F,
                    single_packet=SP_PKT,
                    queue_num=m,
                )
                return g2
            else:
                nc.gpsimd.dma_gather(
                    out_ap=g[:, :nb, :],
                    in_ap=AP(x_h, m * CHUNK * F, [(F, CHUNK), (1, F)]),
                    idxs_ap=idx_sb[
                        :, m * (NIDX // 16) + OFFS[t] * 8:
                           m * (NIDX // 16) + OFFS[t] * 8 + n_idx // 16
                    ],
                    num_idxs=n_idx,
                    num_idxs_reg=n_idx,
                    elem_size=F,
                    single_packet=SP_PKT,
                    queue_num=m,
                )
            g_tiles[(m, t)] = g
            return g

        if REPS > 1:
            with tc.For_i(0, REPS, 1):
                body()
        else:
            body()

    nc.compile()
    _PROGRAM_CACHE[key] = nc
    return nc


def _prepare(x, W, bias, edge_src, edge_dst, edge_weight, idx_keep):
    q = Q
    while True:
        pre = _preprocess(edge_src, edge_dst, edge_weight, idx_keep, q)
        if pre is not None:
            break
        q += 1
    idxbuf, dstv, ewv, metas, W_CAP, NBLK, CSB = pre
    nc = _build_program(W_CAP, q, NBLK, CSB)

    xf = np.ascontiguousarray(np.asarray(x, dtype=np.float32))
    x16 = xf.astype(ml_dtypes.bfloat16) if GDT == "bf16" else xf
    wt = np.ascontiguousarray(
        np.asarray(W, dtype=np.float32).T
    ).astype(ml_dtypes.bfloat16)
    biasv = np.ascontiguousarray(np.asarray(bias, dtype=np.float32).reshape(P, 1))
    dstv32, ewv32 = dstv, ewv
    if STMODE == "batch":
        dstv = dstv.astype(ml_dtypes.bfloat16)
        ewv = ewv.astype(ml_dtypes.bfloat16)
    in_maps = [
        {
            "x16": x16,
            "idx": idxbuf[c],
            "dstv": dstv[c],
            "ewv": ewv[c],
            "wt": wt,
            "biasv": biasv,
        }
        for c in range(NC)
    ]
    if ACTSPLIT:
        for c in range(NC):
            in_maps[c]["dstn"] = -dstv32[c]
            in_maps[c]["ewf"] = ewv32[c]
            in_maps[c]["ewn"] = -ewv32[c]
    return nc, in_maps, metas


def _unpack(results, metas):
    out = np.empty((N_NODES, F), np.float32)
    for c in range(NC):
        o = results[c]["out"]  # [P, W_CAP*P], rows = out features
        base = c * NPC
        for w, win in enumerate(metas[c]):
            if isinstance(win, tuple):
                s, e = win
                out[base + s:base + e, :] = o[:, w * P:w * P + (e - s)].T
            else:  # node array in placement (column) order
                out[base + win, :] = o[:, w * P:w * P + len(win)].T
    return out


def kernel(x, W, bias, edge_src, edge_dst, edge_weight, idx_keep):
    nc, in_maps, metas = _prepare(
        x, W, bias, edge_src, edge_dst, edge_weight, idx_keep
    )
    res = run_bass_kernel_spmd(nc, in_maps, list(range(NC)))
    return _unpack(res.results, metas)


# --- helpers for test.py (not used by the grading harness) ---------------

def run_traced(x, W, bias, edge_src, edge_dst, edge_weight, idx_keep):
    nc, in_maps, metas = _prepare(
        x, W, bias, edge_src, edge_dst, edge_weight, idx_keep
    )
    res = run_bass_kernel_spmd(nc, in_maps, list(range(NC)), trace=True)
    return _unpack(res.results, metas), res


def run_sim(x, W, bias, edge_src, edge_dst, edge_weight, idx_keep, cores=(0,)):
    from concourse.bass_interp import CoreSim

    nc, in_maps, metas = _prepare(
        x, W, bias, edge_src, edge_dst, edge_weight, idx_keep
    )
    results = []
    for c in cores:
        sim = CoreSim(nc)
        for k, v in in_maps[c].items():
            sim.tensor(k)[:] = v
        sim.simulate()
        results.append({"out": sim.tensor("out").copy()})
    return results, metas, in_maps


# revision 69
# speedup vs baseline: 4.2365x; 1.0527x over previous
"""GCNConvRnd kernel for 8 Trainium2 NeuronCores (Bass/Tile) — v2 (bf16).

out = segment_sum((x @ W.T)[src[keep]] * ew[keep], dst[keep], N) + bias

Strategy (dst-sharded, W applied after aggregation):
  * kept edges deduped on host (idx_keep samples WITH replacement: ~31%
    duplicates merge into edge-weight multiplicity)
  * x cast to bf16 on host and shipped as the gather source: halves gather
    bytes and makes every matmul a 1-cycle/row bf16 matmul
  * nodes / output sharded 12500 per core; kept edges partitioned by dst
  * each core gathers x16[src] rows with GPSIMD dma_gather (int16 indices,
    4 chunks of 25000 rows so local index fits int16)
  * edges sorted by dst, packed into windows of <=128 consecutive dst nodes,
    each (window, chunk) padded to exactly Q blocks of 128 edges -> fully
    static SPMD-uniform program (B = 4*Q blocks per window)
  * per 128-edge block: S_T[e, d] = (iota[e,d] == dstv[e]) * ew[e] via one
    DVE tensor_scalar (bf16 in/out -> 2x mode), then PE bf16 matmul
    psum[f, d] += G[e, f].T @ S_T[e, d] accumulated over the window's blocks
  * per window: PSUM -> bf16 SBUF accumulator column
  * epilogue: out2 = W @ acc (+bias) in 512-wide bf16 matmuls, DMA to HBM
  * host unpacks windows back to node order
"""

import os
import numpy as np
import ml_dtypes
from contextlib import ExitStack

import concourse.bass as bass
from concourse.bass import AP
import concourse.mybir as mybir
import concourse.tile as tile
from concourse import bacc
from concourse.bass_utils import run_bass_kernel_spmd

N_NODES = 100000
N_EDGES = 1600000
F = 128
P = 128
NC = 8
NPC = N_NODES // NC      # 12500 nodes per core
NCHUNKS = 4
CHUNK = N_NODES // NCHUNKS  # 25000 rows per src chunk (int16-addressable)

# Tunables
Q = int(os.environ.get("GCN_Q", "2"))        # blocks per (window, chunk)
NB = int(os.environ.get("GCN_NB", "25"))     # class-stream blocks per gather
G_BUFS = int(os.environ.get("GCN_GBUFS", "4"))
S_BUFS = int(os.environ.get("GCN_SBUFS", "8"))
PS_BUFS = int(os.environ.get("GCN_PSBUFS", "6"))
REPS = int(os.environ.get("GCN_REPS", "1"))  # in-NEFF repetitions (timing)
SKIP = os.environ.get("GCN_SKIP", "")        # '', 'gather', or 'compute'
GDT = os.environ.get("GCN_GDT", "bf16")      # gather dtype: 'bf16' | 'f32'
SP_PKT = os.environ.get("GCN_SP", "0") == "1"  # dma_gather single_packet
HALF = os.environ.get("GCN_HALF", "0") == "1"  # diag: half descs, 2x elem
STMODE = os.environ.get("GCN_STMODE", "batch")  # 'batch' | 'scalar'
# every ACTSPLIT-th window builds S_T on the Activation engine (0 = off)
ACTSPLIT = int(os.environ.get("GCN_ACTSPLIT", "3"))
# d-major strip layout: packed last dims -> DVE 2x mode eligible
ST2X = os.environ.get("GCN_ST2X", "0") == "1"
# window packing: 'greedy' contiguous scan | 'ff' first-fit node sets
PACK = os.environ.get("GCN_PACK", "greedy")
# size of the first gather call per queue (0 = uniform NB); small first
# call lets compute start sooner
FIRSTNB = int(os.environ.get("GCN_FIRSTNB", "0"))
# dst-window width in nodes (columns of the one-hot scatter matmul)
WD = int(os.environ.get("GCN_WD", "128"))

f32 = mybir.dt.float32
bf16 = mybir.dt.bfloat16
i16 = mybir.dt.int16

_PROGRAM_CACHE: dict = {}


def _dedup(edge_src, edge_dst, edge_weight, idx_keep):
    """Merge duplicate draws of the same edge into a weight multiplier."""
    cnt = np.bincount(np.asarray(idx_keep), minlength=N_EDGES)
    sel = np.nonzero(cnt)[0]
    src = np.asarray(edge_src)[sel].astype(np.int64)
    dst = np.asarray(edge_dst)[sel].astype(np.int64)
    ew = np.asarray(edge_weight)[sel].astype(np.float32) * cnt[sel].astype(
        np.float32
    )
    return src, dst, ew


def _preprocess(edge_src, edge_dst, edge_weight, idx_keep, q):
    """Shard kept (deduped) edges by dst, pack dst windows under per-chunk
    quotas, and emit the static device layout.

    Returns None if quotas are infeasible; caller bumps q.
    """
    if PACK == "ff" and WD != P:
        raise ValueError("ff packing not wired for WD != 128")
    src, dst, ew = _dedup(edge_src, edge_dst, edge_weight, idx_keep)
    order = np.argsort(dst, kind="stable")
    src, dst, ew = src[order], dst[order], ew[order]
    core_bounds = np.searchsorted(dst, np.arange(NC + 1) * NPC)

    QCAP = q * P
    B = NCHUNKS * q  # compute blocks per window
    percore = []
    for c in range(NC):
        lo, hi = int(core_bounds[c]), int(core_bounds[c + 1])
        dl = dst[lo:hi] - c * NPC
        ch = src[lo:hi] // CHUNK
        cnts = np.bincount(dl * NCHUNKS + ch, minlength=NPC * NCHUNKS).reshape(
            NPC, NCHUNKS
        )
        if PACK == "ff":
            if (cnts > QCAP).any():
                return None
            # first-fit decreasing over 4-dim chunk-count vectors
            order_n = np.argsort(-cnts.max(axis=1), kind="stable")
            wsums = np.zeros((0, NCHUNKS), np.int64)
            wcnt = np.zeros(0, np.int64)
            win_of = np.zeros(NPC, np.int64)
            pos_in = np.zeros(NPC, np.int64)
            wins = []
            for n in order_n:
                row = cnts[n]
                ok = (wcnt < P) & (wsums + row <= QCAP).all(axis=1)
                if ok.any():
                    # best-fit: fullest feasible window
                    load = wsums.sum(axis=1) + wcnt  # count as tiebreak
                    wi = int(np.argmax(np.where(ok, load, -1)))
                else:
                    wi = -1
                if wi < 0:
                    wi = len(wins)
                    wins.append([])
                    wsums = np.vstack([wsums, np.zeros((1, NCHUNKS),
                                                       np.int64)])
                    wcnt = np.append(wcnt, 0)
                win_of[n] = wi
                pos_in[n] = wcnt[wi]
                wins[wi].append(n)
                wsums[wi] += row
                wcnt[wi] += 1
            wins = [np.asarray(wn, np.int64) for wn in wins]
        else:
            wins = []
            n = 0
            cl = cnts.tolist()
            while n < NPC:
                s = n
                acc = [0, 0, 0, 0]
                while n < NPC and (n - s) < WD:
                    row = cl[n]
                    if any(acc[m] + row[m] > QCAP for m in range(NCHUNKS)):
                        break
                    for m in range(NCHUNKS):
                        acc[m] += row[m]
                    n += 1
                if n == s:
                    return None
                wins.append((s, n))
        percore.append((lo, hi, dl, ch, cnts, wins))

    W_CAP = max(len(pc[5]) for pc in percore)
    EPI_EVERY = 512 // WD
    W_CAP = -(-W_CAP // EPI_EVERY) * EPI_EVERY  # epilogue chunks of 512
    NBLK = W_CAP * B             # compute blocks per core
    CSB = W_CAP * q              # class-stream blocks per chunk per core
    NIDX = CSB * P               # idxs per chunk stream

    idxbuf = np.zeros((NC, P, NCHUNKS * (NIDX // 16)), np.int16)
    dstv = np.zeros((NC, NBLK * P), np.float32)
    ewv = np.zeros((NC, NBLK * P), np.float32)
    metas = []
    for c, (lo, hi, dl, ch, cnts, wins) in enumerate(percore):
        ne = hi - lo
        key = dl * NCHUNKS + ch
        o2 = np.argsort(key, kind="stable")
        src_l = src[lo:hi][o2] - ch[o2] * CHUNK
        ew_l = ew[lo:hi][o2]
        dl_l = dl[o2]
        ch_l = ch[o2]
        S = np.zeros(NPC * NCHUNKS + 1, np.int64)
        np.cumsum(cnts.ravel(), out=S[1:])
        segstart = S[dl_l * NCHUNKS + ch_l]
        rank = np.arange(ne) - segstart
        if PACK == "ff":
            win_of = np.zeros(NPC, np.int64)
            colv = np.zeros(NPC, np.int64)
            off_seg = np.zeros((NPC, NCHUNKS), np.int64)
            for w, wn in enumerate(wins):
                win_of[wn] = w
                colv[wn] = np.arange(len(wn))
                pre = np.zeros((len(wn), NCHUNKS), np.int64)
                np.cumsum(cnts[wn][:-1], axis=0, out=pre[1:])
                off_seg[wn] = pre
            wj = win_of[dl_l]
            off_in_seg = off_seg[dl_l, ch_l]
            colw = colv[dl_l]
        else:
            Cn = np.zeros((NPC + 1, NCHUNKS), np.int64)
            np.cumsum(cnts, axis=0, out=Cn[1:])
            win_of = np.zeros(NPC, np.int64)
            wstart = np.zeros(NPC, np.int64)
            for w, (s, e) in enumerate(wins):
                win_of[s:e] = w
                wstart[s:e] = s
            wj = win_of[dl_l]
            swj = wstart[dl_l]
            off_in_seg = Cn[dl_l, ch_l] - Cn[swj, ch_l]
            colw = dl_l - swj
        slot = wj * (q * P) + off_in_seg + rank
        cols = slot // 16
        parts = slot % 16
        base_cols = ch_l * (NIDX // 16)
        flat16 = np.zeros((16, NCHUNKS * (NIDX // 16)), np.int16)
        flat16[parts, base_cols + cols] = src_l.astype(np.int16)
        idxbuf[c] = np.tile(flat16, (8, 1))
        cb = slot // P
        pp = slot % P
        qq = cb % q
        blk = wj * B + ch_l * q + qq
        dv = np.zeros((NBLK, P), np.float32)
        ev = np.zeros((NBLK, P), np.float32)
        dv[blk, pp] = colw.astype(np.float32)
        ev[blk, pp] = ew_l
        dstv[c] = dv.reshape(-1)
        ewv[c] = ev.reshape(-1)
        metas.append(wins)

    dstv = np.ascontiguousarray(dstv.reshape(NC, NBLK, P).transpose(0, 2, 1))
    ewv = np.ascontiguousarray(ewv.reshape(NC, NBLK, P).transpose(0, 2, 1))
    return idxbuf, dstv, ewv, metas, W_CAP, NBLK, CSB


def _build_program(W_CAP, q, NBLK, CSB):
    key = (W_CAP, q, NBLK, CSB, NB, G_BUFS, S_BUFS, PS_BUFS, REPS, SKIP, GDT,
           SP_PKT, HALF, STMODE, ACTSPLIT, ST2X, FIRSTNB, WD)
    if key in _PROGRAM_CACHE:
        return _PROGRAM_CACHE[key]

    B = NCHUNKS * q
    NIDX = CSB * P
    IDXCOLS = NCHUNKS * (NIDX // 16)

    nc = bacc.Bacc(
        "TRN2",
        target_bir_lowering=False,
        debug=False,
        enable_asserts=False,
        num_devices=NC,
        num_swdge_queues=4,
    )
    gdt = bf16 if GDT == "bf16" else f32
    x_h = nc.dram_tensor("x16", [N_NODES, F], gdt, kind="ExternalInput")
    idx_d = nc.dram_tensor("idx", [P, IDXCOLS], i16, kind="ExternalInput").ap()
    sdt = bf16 if STMODE == "batch" else f32
    dstv_d = nc.dram_tensor("dstv", [P, NBLK], sdt, kind="ExternalInput").ap()
    ewv_d = nc.dram_tensor("ewv", [P, NBLK], sdt, kind="ExternalInput").ap()
    if ACTSPLIT:
        # fp32 per-partition scale/bias columns for the ACT-engine one-hot:
        # s_t = Relu(ew - ew*|iota - dst|)
        dstn_d = nc.dram_tensor("dstn", [P, NBLK], f32, kind="ExternalInput").ap()
        ewf_d = nc.dram_tensor("ewf", [P, NBLK], f32, kind="ExternalInput").ap()
        ewn_d = nc.dram_tensor("ewn", [P, NBLK], f32, kind="ExternalInput").ap()
    wt_d = nc.dram_tensor("wt", [P, P], bf16, kind="ExternalInput").ap()  # W.T
    bias_d = nc.dram_tensor("biasv", [P, 1], f32, kind="ExternalInput").ap()
    out_d = nc.dram_tensor("out", [P, W_CAP * WD], f32, kind="ExternalOutput").ap()

    iota_np = np.broadcast_to(
        np.arange(WD, dtype=np.float32), (P, WD)
    ).astype(ml_dtypes.bfloat16)
    iota_d = nc.inline_tensor(iota_np, "iota").ap()
    if ST2X:
        # iota repeated B-wise: col c = d*B + bl -> value d
        iotar_np = np.broadcast_to(
            np.repeat(np.arange(P, dtype=np.float32), B), (P, B * P)
        ).astype(ml_dtypes.bfloat16)
        iotar_d = nc.inline_tensor(iotar_np, "iotar").ap()

    NOCHUNK = (W_CAP * WD) // 512
    EPI = 512 // WD
    # gather call sizes per chunk stream (small first call -> early compute)
    SIZES = []
    left = CSB
    if FIRSTNB and FIRSTNB < min(NB, CSB):
        SIZES.append(FIRSTNB)
        left -= FIRSTNB
    while left > 0:
        s = min(NB, left)
        SIZES.append(s)
        left -= s
    OFFS = [0]
    for s in SIZES:
        OFFS.append(OFFS[-1] + s)
    call_of = []
    col_of = []
    for t, s in enumerate(SIZES):
        call_of += [t] * s
        col_of += list(range(s))
    NGC = len(SIZES)  # gather calls per chunk

    with tile.TileContext(nc) as tc, ExitStack() as ctx:
        const = ctx.enter_context(tc.tile_pool(name="const", bufs=1))
        gpools = [
            ctx.enter_context(tc.tile_pool(name=f"g{m}", bufs=G_BUFS))
            for m in range(NCHUNKS)
        ]
        spool = ctx.enter_context(tc.tile_pool(name="s", bufs=S_BUFS))
        pspool = ctx.enter_context(tc.tile_pool(name="ps", bufs=PS_BUFS, space="PSUM"))
        ps2pool = ctx.enter_context(tc.tile_pool(name="ps2", bufs=2, space="PSUM"))
        stpool = ctx.enter_context(tc.tile_pool(name="st", bufs=2))

        iota_sb = const.tile([P, WD], bf16)
        nc.sync.dma_start(out=iota_sb[:], in_=iota_d[:])
        if ST2X:
            iotar_sb = const.tile([P, B * P], bf16)
            nc.sync.dma_start(out=iotar_sb[:], in_=iotar_d[:])
        wt_sb = const.tile([P, P], bf16)
        nc.sync.dma_start(out=wt_sb[:], in_=wt_d[:])
        bias_sb = const.tile([P, 1], f32)
        nc.sync.dma_start(out=bias_sb[:], in_=bias_d[:])
        idx_sb = const.tile([P, IDXCOLS], i16)
        for m in range(NCHUNKS):  # per-chunk loads: first gathers start sooner
            nc.sync.dma_start(
                out=idx_sb[:, m * (NIDX // 16):(m + 1) * (NIDX // 16)],
                in_=idx_d[:, m * (NIDX // 16):(m + 1) * (NIDX // 16)],
            )
        dstv_sb = const.tile([P, NBLK], sdt)
        nc.sync.dma_start(out=dstv_sb[:], in_=dstv_d[:])
        ewv_sb = const.tile([P, NBLK], sdt)
        nc.sync.dma_start(out=ewv_sb[:], in_=ewv_d[:])
        if ACTSPLIT:
            dstn_sb = const.tile([P, NBLK], f32)
            nc.sync.dma_start(out=dstn_sb[:], in_=dstn_d[:])
            ewf_sb = const.tile([P, NBLK], f32)
            nc.sync.dma_start(out=ewf_sb[:], in_=ewf_d[:])
            ewn_sb = const.tile([P, NBLK], f32)
            nc.sync.dma_start(out=ewn_sb[:], in_=ewn_d[:])
        acc = const.tile([P, W_CAP * WD], bf16)

        g_tiles = {}

        def body():
            g_tiles.clear()
            if SKIP == "compute":
                for t in range(NGC):  # t-major: keep all 4 queues busy
                    for m in range(NCHUNKS):
                        ensure_gather(m, t)
                return
            for w in range(W_CAP):
                ps = pspool.tile([P, WD], f32, space="PSUM")
                act_win = ACTSPLIT and (w % ACTSPLIT == ACTSPLIT - 1)
                if act_win:
                    pass  # per-block ACT build below
                elif STMODE == "batch":
                    # one S_T strip for all B blocks of the window:
                    # tmp = (iota bcast) == (dstv bcast); s_t = tmp * ew
                    s_t = spool.tile([P, B * WD], bf16)
                    tmp = spool.tile([P, B * WD], bf16)
                    if ST2X:
                        # d-major cols (c = d*B + bl): all last dims packed
                        in0 = iotar_sb[:]
                        dstv_b = AP(
                            dstv_sb.tensor,
                            dstv_sb[:, w * B:(w + 1) * B].offset,
                            [dstv_sb[:].ap[0], (0, P), (1, B)],
                        )
                        ewv_b = AP(
                            ewv_sb.tensor,
                            ewv_sb[:, w * B:(w + 1) * B].offset,
                            [ewv_sb[:].ap[0], (0, P), (1, B)],
                        )
                    else:
                        in0 = AP(
                            iota_sb.tensor,
                            iota_sb[:].offset,
                            [iota_sb[:].ap[0], (0, B), (1, WD)],
                        )
                        dstv_b = AP(
                            dstv_sb.tensor,
                            dstv_sb[:, w * B:(w + 1) * B].offset,
                            [dstv_sb[:].ap[0], (1, B), (0, WD)],
                        )
                        ewv_b = AP(
                            ewv_sb.tensor,
                            ewv_sb[:, w * B:(w + 1) * B].offset,
                            [ewv_sb[:].ap[0], (1, B), (0, WD)],
                        )
                    nc.vector.tensor_tensor(
                        out=tmp[:], in0=in0, in1=dstv_b,
                        op=mybir.AluOpType.is_equal,
                    )
                    nc.vector.tensor_tensor(
                        out=s_t[:], in0=tmp[:], in1=ewv_b,
                        op=mybir.AluOpType.mult,
                    )
                for m in range(NCHUNKS):
                    for qq in range(q):
                        blk = w * B + m * q + qq
                        bl = m * q + qq  # block index within window
                        cb = w * q + qq
                        t, col = call_of[cb], col_of[cb]
                        g = ensure_gather(m, t)
                        if act_win:
                            u = spool.tile([P, WD], bf16)
                            nc.scalar.activation(
                                out=u[:], in_=iota_sb[:],
                                func=mybir.ActivationFunctionType.Abs,
                                bias=dstn_sb[:, blk:blk + 1], scale=1.0,
                            )
                            s1 = spool.tile([P, WD], bf16)
                            nc.scalar.activation(
                                out=s1[:], in_=u[:],
                                func=mybir.ActivationFunctionType.Relu,
                                bias=ewf_sb[:, blk:blk + 1],
                                scale=ewn_sb[:, blk:blk + 1],
                            )
                            rhs = s1[:]
                        elif STMODE == "batch":
                            if ST2X:
                                # column d of block bl lives at c = d*B + bl
                                rhs = AP(
                                    s_t.tensor,
                                    s_t[:].offset + bl,
                                    [s_t[:].ap[0], (B, P)],
                                )
                            else:
                                rhs = s_t[:, bl * WD:(bl + 1) * WD]
                        else:
                            s1 = spool.tile([P, WD], bf16)
                            nc.vector.tensor_scalar(
                                out=s1[:],
                                in0=iota_sb[:],
                                scalar1=dstv_sb[:, blk:blk + 1],
                                scalar2=ewv_sb[:, blk:blk + 1],
                                op0=mybir.AluOpType.is_equal,
                                op1=mybir.AluOpType.mult,
                            )
                            rhs = s1[:]
                        first = m == 0 and qq == 0
                        last = m == NCHUNKS - 1 and qq == q - 1
                        nc.tensor.matmul(
                            out=ps[:],
                            lhsT=g[:, col, :],
                            rhs=rhs,
                            start=first,
                            stop=last,
                        )
                if act_win:  # keep ACT free on its windows
                    nc.vector.tensor_copy(
                        out=acc[:, w * WD:(w + 1) * WD], in_=ps[:]
                    )
                else:
                    nc.scalar.copy(out=acc[:, w * WD:(w + 1) * WD], in_=ps[:])
                if (w + 1) % EPI == 0:  # stream the epilogue as acc fills
                    epi_chunk((w + 1) // EPI - 1)

        def epilogue():
            for cix in range(NOCHUNK):
                epi_chunk(cix)

        def epi_chunk(cix):
                ps2 = ps2pool.tile([P, 512], f32, space="PSUM")
                nc.tensor.matmul(
                    out=ps2[:],
                    lhsT=wt_sb[:],
                    rhs=acc[:, cix * 512:(cix + 1) * 512],
                    start=True,
                    stop=True,
                )
                st = stpool.tile([P, 512], f32)
                nc.scalar.add(out=st[:], in_=ps2[:], add=bias_sb[:, 0:1])
                nc.sync.dma_start(out=out_d[:, cix * 512:(cix + 1) * 512], in_=st[:])

        def ensure_gather(m, t):
            if (m, t) in g_tiles:
                return g_tiles[(m, t)]
            nb = SIZES[t]
            n_idx = nb * P
            g = gpools[m].tile([P, nb, F], gdt)
            if SKIP == "gather":
                # sequential-stream stand-in write: keeps the tile written
                # (framework requirement) at streaming DMA cost, no descgen
                nc.sync.dma_start(
                    out=g[:, :nb, :],
                    in_=AP(x_h, m * CHUNK * F,
                           [(F, P), (F * P, nb), (1, F)]),
                )
                g_tiles[(m, t)] = g
                return g
            if HALF:
                # diagnostic only (wrong data): same bytes, half descriptors
                g2 = gpools[m].tile([P, max(nb // 2, 1), 2 * F], gdt)
                g_tiles[(m, t)] = g2
                nc.gpsimd.dma_gather(
                    out_ap=g2[:, :nb // 2, :],
                    in_ap=AP(x_h, 0, [(2 * F, CHUNK), (1, 2 * F)]),
                    idxs_ap=idx_sb[
                        :, m * (NIDX // 16) + OFFS[t] * 8:
                           m * (NIDX // 16) + OFFS[t] * 8 + n_idx // 32
                    ],
                    num_idxs=n_idx // 2,
                    num_idxs_reg=n_idx // 2,
                    elem_size=2 * F,
                    single_packet=SP_PKT,
                    queue_num=m,
                )
                return g2
            else:
                nc.gpsimd.dma_gather(
                    out_ap=g[:, :nb, :],
                    in_ap=AP(x_h, m * CHUNK * F, [(F, CHUNK), (1, F)]),
                    idxs_ap=idx_sb[
                        :, m * (NIDX // 16) + OFFS[t] * 8:
                           m * (NIDX // 16) + OFFS[t] * 8 + n_idx // 16
                    ],
                    num_idxs=n_idx,
                    num_idxs_reg=n_idx,
                    elem_size=F,
                    single_packet=SP_PKT,
                    queue_num=m,
                )
            g_tiles[(m, t)] = g
            return g

        if REPS > 1:
            with tc.For_i(0, REPS, 1):
                body()
        else:
            body()

    nc.compile()
    _PROGRAM_CACHE[key] = nc
    return nc


def _prepare(x, W, bias, edge_src, edge_dst, edge_weight, idx_keep):
    q = Q
    while True:
        pre = _preprocess(edge_src, edge_dst, edge_weight, idx_keep, q)
        if pre is not None:
            break
        q += 1
    idxbuf, dstv, ewv, metas, W_CAP, NBLK, CSB = pre
    nc = _build_program(W_CAP, q, NBLK, CSB)

    xf = np.ascontiguousarray(np.asarray(x, dtype=np.float32))
    x16 = xf.astype(ml_dtypes.bfloat16) if GDT == "bf16" else xf
    wt = np.ascontiguousarray(
        np.asarray(W, dtype=np.float32).T
    ).astype(ml_dtypes.bfloat16)
    biasv = np.ascontiguousarray(np.asarray(bias, dtype=np.float32).reshape(P, 1))
    dstv32, ewv32 = dstv, ewv
    if STMODE == "batch":
        dstv = dstv.astype(ml_dtypes.bfloat16)
        ewv = ewv.astype(ml_dtypes.bfloat16)
    in_maps = [
        {
            "x16": x16,
            "idx": idxbuf[c],
            "dstv": dstv[c],
            "ewv": ewv[c],
            "wt": wt,
            "biasv": biasv,
        }
        for c in range(NC)
    ]
    if ACTSPLIT:
        for c in range(NC):
            in_maps[c]["dstn"] = -dstv32[c]
            in_maps[c]["ewf"] = ewv32[c]
            in_maps[c]["ewn"] = -ewv32[c]
    return nc, in_maps, metas


def _unpack(results, metas):
    out = np.empty((N_NODES, F), np.float32)
    for c in range(NC):
        o = results[c]["out"]  # [P, W_CAP*P], rows = out features
        base = c * NPC
        for w, win in enumerate(metas[c]):
            if isinstance(win, tuple):
                s, e = win
                out[base + s:base + e, :] = o[:, w * WD:w * WD + (e - s)].T
            else:  # node array in placement (column) order
                out[base + win, :] = o[:, w * WD:w * WD + len(win)].T
    return out


def kernel(x, W, bias, edge_src, edge_dst, edge_weight, idx_keep):
    nc, in_maps, metas = _prepare(
        x, W, bias, edge_src, edge_dst, edge_weight, idx_keep
    )
    res = run_bass_kernel_spmd(nc, in_maps, list(range(NC)))
    return _unpack(res.results, metas)


# --- helpers for test.py (not used by the grading harness) ---------------

def run_traced(x, W, bias, edge_src, edge_dst, edge_weight, idx_keep):
    nc, in_maps, metas = _prepare(
        x, W, bias, edge_src, edge_dst, edge_weight, idx_keep
    )
    res = run_bass_kernel_spmd(nc, in_maps, list(range(NC)), trace=True)
    return _unpack(res.results, metas), res


def run_sim(x, W, bias, edge_src, edge_dst, edge_weight, idx_keep, cores=(0,)):
    from concourse.bass_interp import CoreSim

    nc, in_maps, metas = _prepare(
        x, W, bias, edge_src, edge_dst, edge_weight, idx_keep
    )
    results = []
    for c in cores:
        sim = CoreSim(nc)
        for k, v in in_maps[c].items():
            sim.tensor(k)[:] = v
        sim.simulate()
        results.append({"out": sim.tensor("out").copy()})
    return results, metas, in_maps
